# revision 27
# baseline (speedup 1.0000x reference)
"""Trainium2 Bass kernel for a dense pre-LN transformer block.

Sharding: 8 cores = 4 batches x 2 sequence-halves (zigzag query blocks).
Each core handles one batch element; K/V are computed redundantly for the
full sequence on both cores of a batch (cheaper than collectives), and each
core computes attention + proj + FFN for 1024 of the 2048 query tokens.

To keep the SPMD instruction stream identical across cores, each core's
tokens are host-side permuted to [own_blockA; own_blockB; rest] and all
causal-validity variation is carried in per-core mask data (triangular
tiles for diagonal blocks, per-partition 0/1 scalars for whole chunks).

All matmuls run as float32r (FP22, full PE rate); softmax/LN in fp32.
"""

import contextlib

import numpy as np

from concourse import bass, bacc, tile, mybir
from concourse.bass_utils import run_bass_kernel_spmd

F32 = mybir.dt.float32
F32R = mybir.dt.float32r
BF16 = mybir.dt.bfloat16

B, T, D = 4, 2048, 1024
H, HD = 16, 64
DFF = 4 * D
EPS = 1e-5
N_CORES = 8

FULL_CFG = dict(D=1024, H=16, T=2048, QB=512, DFF=4096, NG=4)
SMALL_CFG = dict(D=256, H=4, T=512, QB=128, DFF=512, NG=2)


def derive(cfg):
    c = dict(cfg)
    c["DC"] = cfg["D"] // 128            # d-chunks
    c["FC"] = cfg["H"] * HD // 128       # feature chunks (head pairs)
    c["FCP"] = 2                         # f-chunks per pass
    c["NPASS"] = c["FC"] // c["FCP"]
    c["S"] = cfg["T"] // 128             # key chunks
    c["QBC"] = cfg["QB"] // 128          # chunks per query block
    c["NT"] = cfg["QB"]                  # moving-dim tile (== query block)
    c["TOWN"] = 2 * cfg["QB"]            # tokens owned per core
    c["TOC"] = c["TOWN"] // 128
    c["NO"] = min(512, cfg["D"])
    c["OC"] = cfg["D"] // c["NO"]        # dout chunks of <=512
    c["GFC"] = (cfg["DFF"] // cfg["NG"]) // 128  # f-chunks per FFN group
    c["KTB"] = cfg["T"] // c["NT"]       # t-blocks for k over full T
    return c


def build(cfg):
    """Emit the bass program for one core. Returns nc."""
    c = derive(cfg)
    Dm, Tf, DFFm, NG = cfg["D"], cfg["T"], cfg["DFF"], cfg["NG"]
    DC, FC, FCP, NPASS = c["DC"], c["FC"], c["FCP"], c["NPASS"]
    S, QBC, NT, TOWN, TOC = c["S"], c["QBC"], c["NT"], c["TOWN"], c["TOC"]
    OC, NO, GFC, KTB = c["OC"], c["NO"], c["GFC"], c["KTB"]
    HDf = HD  # 64

    nc = bacc.Bacc("TRN2", target_bir_lowering=False, debug=False)

    # ---- DRAM I/O ----
    x_d = nc.dram_tensor("x", [Tf, Dm], F32, kind="ExternalInput")
    wq_d = nc.dram_tensor("wq", [NPASS, DC, 128, FCP * 128], F32,
                          kind="ExternalInput")
    wk_d = nc.dram_tensor("wk", [NPASS, DC, 128, FCP * 128], F32,
                          kind="ExternalInput")
    wv_d = nc.dram_tensor("wv", [NPASS, DC, 128, FCP * 130], F32,
                          kind="ExternalInput")
    bq_d = nc.dram_tensor("bq", [FC, 128, 1], F32, kind="ExternalInput")
    bk_d = nc.dram_tensor("bk", [FC, 128, 1], F32, kind="ExternalInput")
    bv_d = nc.dram_tensor("bv", [NPASS, 1, FCP * 130], F32,
                          kind="ExternalInput")
    wo_d = nc.dram_tensor("wo", [FC * 128, Dm], F32, kind="ExternalInput")
    bo_d = nc.dram_tensor("bo", [1, Dm], F32, kind="ExternalInput")
    w1_d = nc.dram_tensor("w1", [NG, DC, 128, DFFm // NG], F32,
                          kind="ExternalInput")
    b1_d = nc.dram_tensor("b1", [DFFm // 128, 128, 1], F32, kind="ExternalInput")
    w2_d = nc.dram_tensor("w2", [DFFm, Dm], F32, kind="ExternalInput")
    b2_d = nc.dram_tensor("b2", [128, Dm], F32, kind="ExternalInput")
    tri_d = nc.dram_tensor("tri", [QBC, 128, NT], F32, kind="ExternalInput")
    cm_d = nc.dram_tensor("cm", [2 * QBC, 128, 1], F32, kind="ExternalInput")
    idn_d = nc.dram_tensor("ident", [128, 128], F32, kind="ExternalInput")
    one_d = nc.dram_tensor("ones", [128, 1], F32, kind="ExternalInput")
    zro_d = nc.dram_tensor("zeros", [128, 1], F32, kind="ExternalInput")
    out_d = nc.dram_tensor("out", [TOWN, Dm], F32, kind="ExternalOutput")
    x1s_d = nc.dram_tensor("x1s", [TOWN, Dm], F32)  # internal scratch

    xr = x_d.ap().rearrange("(n p) d -> n p d", p=128)
    x1r = x1s_d.ap().rearrange("(n p) d -> n p d", p=128)
    outr = out_d.ap().rearrange("(n p) d -> n p d", p=128)

    with tile.TileContext(nc) as tc, contextlib.ExitStack() as top:
        cpool = top.enter_context(tc.tile_pool(name="const", bufs=1))
        ident = cpool.tile([128, 128], F32, name="ident", tag="ident")
        nc.sync.dma_start(ident[:], idn_d.ap())
        onesc = cpool.tile([128, 1], F32R, name="onesc", tag="onesc")
        nc.sync.dma_start(onesc[:], one_d.ap().bitcast(F32R))
        cms = cpool.tile([128, 2 * QBC], F32, name="cms", tag="cms")
        for i in range(2 * QBC):
            nc.gpsimd.dma_start(cms[:, i:i + 1], cm_d.ap()[i])
        zbias = cpool.tile([128, 1], F32, name="zbias", tag="zbias")
        nc.gpsimd.dma_start(zbias[:], zro_d.ap())

        ctx_stack = contextlib.ExitStack()
        ctxp = ctx_stack.enter_context(tc.tile_pool(name="ctxTp", bufs=1))
        ctxT = [ctxp.tile([128, TOWN], F32R, name=f"ctxT{fc}", tag=f"ctxT{fc}")
                for fc in range(FC)]

        hT_stack = contextlib.ExitStack()
        hp = hT_stack.enter_context(tc.tile_pool(name="hTp", bufs=1))
        hT = [hp.tile([128, Tf], F32R, name=f"hT{dc}", tag=f"hT{dc}")
              for dc in range(DC)]
        trip = hT_stack.enter_context(tc.tile_pool(name="trip", bufs=1))
        tri = []
        for i in range(QBC):
            m = trip.tile([128, NT], F32R, name=f"tri{i}", tag=f"tri{i}")
            nc.gpsimd.dma_start(m[:], tri_d.ap()[i].bitcast(F32R))
            tri.append(m)

        # ---------------- Phase 1: LN1 + transpose -> hT ----------------
        with tc.tile_pool(name="ln1", bufs=4) as lp, \
             tc.tile_pool(name="ln1s", bufs=8) as lsp, \
             tc.tile_pool(name="ln1p", bufs=4, space=bass.MemorySpace.PSUM) as lpp:
            for ti in range(S):
                xt = lp.tile([128, Dm], F32, name="xt", tag="xt")
                nc.sync.dma_start(xt[:], xr[ti])
                nsub = max(1, Dm // 512)
                st6 = lsp.tile([128, nsub, 6], F32, name="st6", tag="st6")
                for sb_i in range(nsub):
                    nc.vector.bn_stats(
                        st6[:, sb_i, :],
                        xt[:, sb_i * (Dm // nsub):(sb_i + 1) * (Dm // nsub)])
                agg = lsp.tile([128, 2], F32, name="agg", tag="agg")
                nc.vector.bn_aggr(agg[:], st6[:])
                veps = lsp.tile([128, 1], F32, name="veps",
                                  tag="veps")
                nc.vector.tensor_scalar_add(veps[:], agg[:, 1:2], EPS)
                std = lsp.tile([128, 1], F32, name="std", tag="std")
                nc.scalar.sqrt(std[:], veps[:])
                rstd = lsp.tile([128, 1], F32, name="rstd", tag="rstd")
                nc.vector.reciprocal(rstd[:], std[:])
                ht = lp.tile([128, Dm], F32, name="ht", tag="ht")
                nc.vector.tensor_scalar(ht[:], xt[:], agg[:, 0:1], rstd[:],
                                        op0=mybir.AluOpType.subtract,
                                        op1=mybir.AluOpType.mult)
                for dc in range(DC):
                    ps = lpp.tile([128, 128], F32, name="tps", tag="tps")
                    nc.tensor.transpose(ps[:], ht[:, dc * 128:(dc + 1) * 128],
                                        ident[:])
                    if dc % 2 == 0:
                        nc.scalar.copy(
                            hT[dc][:, ti * 128:(ti + 1) * 128], ps[:])
                    else:
                        nc.vector.tensor_copy(
                            hT[dc][:, ti * 128:(ti + 1) * 128], ps[:])

        # ---------------- Phase 2: per-pass QKV + attention -------------
        with tc.tile_pool(name="pass_sb", bufs=1) as pp, \
             tc.tile_pool(name="wvres", bufs=1) as wvp, \
             tc.tile_pool(name="expp", bufs=4) as ep, \
             tc.tile_pool(name="zrowp", bufs=2) as zp, \
             tc.tile_pool(name="zbp", bufs=2) as zbp, \
             tc.tile_pool(name="qkvps", bufs=2, space=bass.MemorySpace.PSUM) as qps, \
             tc.tile_pool(name="scps", bufs=2, space=bass.MemorySpace.PSUM) as sps, \
             tc.tile_pool(name="ctxps", bufs=1, space=bass.MemorySpace.PSUM) as cps:
            for p in range(NPASS):
                fcs = [p * FCP + i for i in range(FCP)]
                # --- K^T and Q^T (feature-major) ---
                kT = [pp.tile([128, Tf], F32R, name=f"kT{i}", tag=f"kT{i}")
                      for i in range(FCP)]
                qT = [pp.tile([128, 2 * NT], F32R, name=f"qT{i}", tag=f"qT{i}")
                      for i in range(FCP)]
                bks = []
                bqs = []
                for i, fc in enumerate(fcs):
                    bkt = pp.tile([128, 1], F32, name=f"bk{i}", tag=f"bk{i}")
                    nc.sync.dma_start(bkt[:], bk_d.ap()[fc])
                    bks.append(bkt)
                    bqt = pp.tile([128, 1], F32, name=f"bq{i}", tag=f"bq{i}")
                    nc.sync.dma_start(bqt[:], bq_d.ap()[fc])
                    bqs.append(bqt)
                wkp = []
                wqp = []
                for dc in range(DC):
                    wkt = pp.tile([128, FCP * 128], F32R,
                                  name=f"wkp{dc}", tag=f"wkp{dc}")
                    nc.gpsimd.dma_start(wkt[:], wk_d.ap()[p, dc].bitcast(F32R))
                    wkp.append(wkt)
                    wqt = pp.tile([128, FCP * 128], F32R,
                                  name=f"wqp{dc}", tag=f"wqp{dc}")
                    nc.gpsimd.dma_start(wqt[:], wq_d.ap()[p, dc].bitcast(F32R))
                    wqp.append(wqt)
                for i, fc in enumerate(fcs):
                    ws = slice(i * 128, (i + 1) * 128)
                    for tb in range(KTB):
                        pk = qps.tile([128, NT], F32, name="pk", tag="qkv")
                        for dc in range(DC):
                            nc.tensor.matmul(
                                pk[:], (wkp[dc][:, ws]),
                                (hT[dc][:, tb * NT:(tb + 1) * NT]),
                                start=(dc == 0), stop=(dc == DC - 1))
                        nc.vector.tensor_scalar_add(
                            kT[i][:, tb * NT:(tb + 1) * NT], pk[:], bks[i][:])
                    for tb in range(2):
                        pq = qps.tile([128, NT], F32, name="pq", tag="qkv")
                        for dc in range(DC):
                            nc.tensor.matmul(
                                pq[:], (wqp[dc][:, ws]),
                                (hT[dc][:, tb * NT:(tb + 1) * NT]),
                                start=(dc == 0), stop=(dc == DC - 1))
                        nc.vector.tensor_scalar_add(
                            qT[i][:, tb * NT:(tb + 1) * NT], pq[:], bqs[i][:])
                # --- V (token-major) ---
                bvr = pp.tile([1, FCP * 130], F32, name="bvr", tag="bvr")
                nc.sync.dma_start(bvr[:], bv_d.ap()[p])
                bvb = pp.tile([128, FCP * 130], F32, name="bvb", tag="bvb")
                nc.gpsimd.partition_broadcast(bvb[:], bvr[:])
                wvs = []
                for dc in range(DC):
                    wvt = wvp.tile([128, FCP * 130], F32R,
                                   name=f"wv{dc}", tag=f"wv{dc}")
                    nc.gpsimd.dma_start(wvt[:], wv_d.ap()[p, dc].bitcast(F32R))
                    wvs.append(wvt)
                vt = [pp.tile([128, FCP * 130], F32R, name=f"v{ti}",
                              tag=f"v{ti}") for ti in range(S)]
                for ti in range(S):
                    pv = qps.tile([128, FCP * 130], F32, name="pv", tag="qkv")
                    for dc in range(DC):
                        nc.tensor.matmul(
                            pv[:], (hT[dc][:, ti * 128:(ti + 1) * 128]),
                            (wvs[dc][:]),
                            start=(dc == 0), stop=(dc == DC - 1))
                    nc.vector.tensor_add(vt[ti][:], pv[:], bvb[:])
                # --- attention per head pair ---
                for i, fc in enumerate(fcs):
                    for qb in range(2):
                        if qb == 0:
                            schunks = list(range(S))
                        else:
                            schunks = list(range(QBC, 3 * QBC))
                        ctx_ps = [cps.tile([65, NT], F32, name=f"ctx{hh}",
                                           tag=f"ctx{hh}") for hh in range(2)]
                        nsc = len(schunks)
                        for idx, sc in enumerate(schunks):
                            # mask: (kind, index); kind: 0=none,1=tri,2=scalar
                            if qb == 0:
                                if sc < QBC:
                                    mk = (1, sc)
                                elif sc >= S - QBC:
                                    mk = (2, sc - (S - QBC))
                                else:
                                    mk = (0, 0)
                            else:
                                if sc < 2 * QBC:
                                    mk = (1, sc - QBC)
                                else:
                                    mk = (2, QBC + (sc - 2 * QBC))
                            sps_t = sps.tile([128, 2, NT], F32,
                                             name="sc", tag="sc")
                            e2 = ep.tile([128, 2, NT], F32R, name="e", tag="e")
                            for hh in range(2):
                                rows = slice(hh * HDf, (hh + 1) * HDf)
                                nc.tensor.matmul(
                                    sps_t[:, hh, :],
                                    (kT[i][rows, sc * 128:(sc + 1) * 128]),
                                    (qT[i][rows, qb * NT:(qb + 1) * NT]),
                                    start=True, stop=True,
                                    tile_position=(hh * HDf, 0))
                            ebias = cms[:, mk[1]:mk[1] + 1] \
                                if mk[0] == 2 else zbias[:]
                            nc.scalar.activation(
                                e2[:], sps_t[:],
                                mybir.ActivationFunctionType.Exp,
                                bias=ebias)
                            if mk[0] == 1:
                                nc.vector.tensor_mul(
                                    e2[:], e2[:],
                                    tri[mk[1]][:].unsqueeze(1)
                                    .to_broadcast([128, 2, NT]))
                            for hh in range(2):
                                nc.tensor.matmul(
                                    ctx_ps[hh][:],
                                    (vt[sc][:, (i * 2 + hh) * 65:
                                             (i * 2 + hh) * 65 + 65]),
                                    (e2[:, hh, :]),
                                    start=(idx == 0), stop=(idx == nsc - 1),
                                    skip_group_check=True)
                        for hh in range(2):
                            zrow = zp.tile([1, NT], F32, name="zrow",
                                           tag="zrow")
                            nc.vector.tensor_copy(zrow[:], ctx_ps[hh][64:65, :])
                            rz = zp.tile([1, NT], F32, name="rz", tag="rz")
                            nc.vector.reciprocal(rz[:], zrow[:])
                            zb = zbp.tile([64, NT], F32, name="zb", tag="zb")
                            nc.gpsimd.partition_broadcast(zb[:], rz[:])
                            rows = slice(hh * HDf, (hh + 1) * HDf)
                            nc.vector.tensor_mul(
                                ctxT[fc][rows, qb * NT:(qb + 1) * NT],
                                ctx_ps[hh][0:64, :], zb[:])

        hT_stack.close()

        # ---------------- Phase 3: projection + fused LN2 ---------------
        h2_stack = contextlib.ExitStack()
        h2p = h2_stack.enter_context(tc.tile_pool(name="h2Tp", bufs=1))
        h2T = [h2p.tile([128, TOWN], F32R, name=f"h2T{dc}", tag=f"h2T{dc}")
               for dc in range(DC)]
        with tc.tile_pool(name="proj_sb", bufs=1) as prp, \
             tc.tile_pool(name="proj_x", bufs=3) as pxp, \
             tc.tile_pool(name="proj_o", bufs=3) as pop, \
             tc.tile_pool(name="ln2s", bufs=8) as lsp2, \
             tc.tile_pool(name="ln2h", bufs=3) as lph2, \
             tc.tile_pool(name="projps", bufs=3, space=bass.MemorySpace.PSUM) as pps, \
             tc.tile_pool(name="ln2p", bufs=4, space=bass.MemorySpace.PSUM) as lpp2:
            bo_row = prp.tile([1, Dm], F32, name="bo_row", tag="bo_row")
            nc.sync.dma_start(bo_row[:], bo_d.ap())
            bob = prp.tile([128, Dm], F32, name="bob", tag="bob")
            nc.gpsimd.partition_broadcast(bob[:], bo_row[:])
            wos = []
            for fc in range(FC):
                wot = prp.tile([128, Dm], F32R, name=f"wo{fc}", tag=f"wo{fc}")
                nc.gpsimd.dma_start(
                    wot[:], wo_d.ap()[fc * 128:(fc + 1) * 128, :].bitcast(F32R))
                wos.append(wot)
            for ti in range(TOC):
                xo = pxp.tile([128, Dm], F32, name="xo", tag="xo")
                nc.sync.dma_start(xo[:], xr[ti])
                x1t = pop.tile([128, Dm], F32, name="x1t", tag="x1t")
                for oc in range(OC):
                    ppt = pps.tile([128, NO], F32, name="ppt", tag="ppt")
                    for fc in range(FC):
                        nc.tensor.matmul(
                            ppt[:],
                            (ctxT[fc][:, ti * 128:(ti + 1) * 128]),
                            (wos[fc][:, oc * NO:(oc + 1) * NO]),
                            start=(fc == 0), stop=(fc == FC - 1))
                    cols = slice(oc * NO, (oc + 1) * NO)
                    nc.vector.tensor_add(x1t[:, cols], ppt[:], xo[:, cols])
                    nc.vector.tensor_add(x1t[:, cols], x1t[:, cols],
                                         bob[:, cols])
                nc.sync.dma_start(x1r[ti], x1t[:])
                # fused LN2 on the freshly built x1 tile
                nsub = max(1, Dm // 512)
                st6 = lsp2.tile([128, nsub, 6], F32, name="st6b", tag="st6b")
                for sb_i in range(nsub):
                    nc.vector.bn_stats(
                        st6[:, sb_i, :],
                        x1t[:, sb_i * (Dm // nsub):(sb_i + 1) * (Dm // nsub)])
                agg = lsp2.tile([128, 2], F32, name="aggb", tag="aggb")
                nc.vector.bn_aggr(agg[:], st6[:])
                veps = lsp2.tile([128, 1], F32, name="vepsb", tag="vepsb")
                nc.vector.tensor_scalar_add(veps[:], agg[:, 1:2], EPS)
                std = lsp2.tile([128, 1], F32, name="stdb", tag="stdb")
                nc.scalar.sqrt(std[:], veps[:])
                rstd = lsp2.tile([128, 1], F32, name="rstdb", tag="rstdb")
                nc.vector.reciprocal(rstd[:], std[:])
                hb = lph2.tile([128, Dm], F32, name="hb", tag="hb")
                nc.vector.tensor_scalar(hb[:], x1t[:], agg[:, 0:1], rstd[:],
                                        op0=mybir.AluOpType.subtract,
                                        op1=mybir.AluOpType.mult)
                for dc in range(DC):
                    ps2 = lpp2.tile([128, 128], F32, name="tps2", tag="tps2")
                    nc.tensor.transpose(ps2[:], hb[:, dc * 128:(dc + 1) * 128],
                                        ident[:])
                    nc.scalar.copy(
                        h2T[dc][:, ti * 128:(ti + 1) * 128], ps2[:])

        # (LN2 is fused into the projection loop above; h2T ready here.)
        if False:
            for ti in range(TOC):
                nsub = max(1, Dm // 512)
                pass

        # ---------------- Phase 5: FFN ----------------------------------
        with tc.tile_pool(name="ffn_sb", bufs=1) as fp, \
             tc.tile_pool(name="ffn_w1", bufs=1) as w1p, \
             tc.tile_pool(name="ffn_w2", bufs=8) as w2p, \
             tc.tile_pool(name="ffn_b1", bufs=4) as b1p, \
             tc.tile_pool(name="ffn_x1", bufs=1) as fxp, \
             tc.tile_pool(name="ffn_out", bufs=2) as fop, \
             tc.tile_pool(name="ffps", bufs=3, space=bass.MemorySpace.PSUM) as fps, \
             tc.tile_pool(name="outps", bufs=3, space=bass.MemorySpace.PSUM) as ops:
            b2b = fp.tile([128, Dm], F32, name="b2b", tag="b2b")
            nc.sync.dma_start(b2b[:], b2_d.ap())
            oacc = [fp.tile([128, Dm], F32, name=f"oacc{ti}", tag=f"oacc{ti}")
                    for ti in range(TOC)]
            ffT = [fp.tile([128, TOWN], F32R, name=f"ffT{j}", tag=f"ffT{j}")
                   for j in range(GFC)]
            for g in range(NG):
                w1g = []
                for dc in range(DC):
                    w1t = w1p.tile([128, DFFm // NG], F32R,
                                   name=f"w1g{dc}", tag=f"w1g{dc}")
                    nc.gpsimd.dma_start(w1t[:], w1_d.ap()[g, dc].bitcast(F32R))
                    w1g.append(w1t)
                for j in range(GFC):
                    gf = g * GFC + j
                    b1t = b1p.tile([128, 1], F32, name="b1t", tag="b1t")
                    nc.gpsimd.dma_start(b1t[:], b1_d.ap()[gf])
                    for tb in range(TOWN // NT):
                        fpt = fps.tile([128, NT], F32, name="fpt", tag="fpt")
                        for dc in range(DC):
                            nc.tensor.matmul(
                                fpt[:], (w1g[dc][:, j * 128:(j + 1) * 128]),
                                (h2T[dc][:, tb * NT:(tb + 1) * NT]),
                                start=(dc == 0), stop=(dc == DC - 1))
                        nc.scalar.activation(
                            ffT[j][:, tb * NT:(tb + 1) * NT], fpt[:],
                            mybir.ActivationFunctionType.Relu,
                            bias=b1t[:])
                w2s = []
                for j in range(GFC):
                    gf = g * GFC + j
                    w2t = w2p.tile([128, Dm], F32R, name="w2t", tag="w2t")
                    nc.gpsimd.dma_start(
                        w2t[:], w2_d.ap()[gf * 128:(gf + 1) * 128, :]
                        .bitcast(F32R))
                    w2s.append(w2t)
                for ti in range(TOC):
                    x1t = None
                    if g == NG - 1:
                        x1t = fxp.tile([128, Dm], F32, name="x1f", tag="x1f")
                        nc.sync.dma_start(x1t[:], x1r[ti])
                    for oc in range(OC):
                        opt = ops.tile([128, NO], F32, name="opt", tag="opt")
                        for j in range(GFC):
                            nc.tensor.matmul(
                                opt[:],
                                (ffT[j][:, ti * 128:(ti + 1) * 128]),
                                (w2s[j][:, oc * NO:(oc + 1) * NO]),
                                start=(j == 0), stop=(j == GFC - 1))
                        cols = slice(oc * NO, (oc + 1) * NO)
                        if g == 0:
                            nc.vector.tensor_copy(oacc[ti][:, cols], opt[:])
                        elif g < NG - 1:
                            nc.vector.tensor_add(oacc[ti][:, cols],
                                                 oacc[ti][:, cols], opt[:])
                        else:
                            nc.vector.tensor_add(oacc[ti][:, cols],
                                                 oacc[ti][:, cols], opt[:])
                            nc.vector.tensor_add(oacc[ti][:, cols],
                                                 oacc[ti][:, cols],
                                                 x1t[:, cols])
                            ot = fop.tile([128, NO], F32, name="ot", tag="ot")
                            nc.vector.tensor_add(ot[:], oacc[ti][:, cols],
                                                 b2b[:, cols])
                            nc.sync.dma_start(outr[ti][:, cols], ot[:])
        h2_stack.close()
        ctx_stack.close()
    nc.compile()
    return nc


# ---------------------------------------------------------------------------
# host-side input preparation
# ---------------------------------------------------------------------------

def prepare_shared(cfg, Wq, Wk, Wv, Wo, bo, W1, b1, W2, b2, g1, be1, g2, be2):
    c = derive(cfg)
    Dm, Hn, DFFm, FC = cfg["D"], cfg["H"], cfg["DFF"], c["FC"]
    scale = 1.0 / np.sqrt(HD)
    wq_f = np.ascontiguousarray(Wq.transpose(1, 0, 2).reshape(Dm, Hn * HD))
    wk_f = np.ascontiguousarray(Wk.transpose(1, 0, 2).reshape(Dm, Hn * HD))
    wv_f = np.ascontiguousarray(Wv.transpose(1, 0, 2).reshape(Dm, Hn * HD))
    wq_e = (g1[:, None] * wq_f) * scale
    wk_e = g1[:, None] * wk_f
    wv_e = g1[:, None] * wv_f
    bq = ((be1 @ wq_f) * scale).reshape(FC, 128, 1)
    bk = (be1 @ wk_f).reshape(FC, 128, 1)
    bv = (be1 @ wv_f).reshape(1, Hn * HD)
    w1_e = g2[:, None] * W1
    b1_e = (b1 + be2 @ W1).reshape(DFFm // 128, 128, 1)
    DC, NPASS, FCP, NG = c["DC"], c["NPASS"], c["FCP"], cfg["NG"]

    def qkv_tile(w):
        # [D, F] -> [NPASS, DC, 128, FCP*128]
        return w.reshape(DC, 128, NPASS, FCP * 128).transpose(2, 0, 1, 3)

    # v weights get a zero column appended per head; its bias is 1.0, so the
    # v tiles come out of the matmul+bias with a built-in ones column that
    # accumulates the softmax normalizer during the ctx matmul.
    nheads = FCP * 2
    wv_r = wv_e.reshape(DC, 128, NPASS, nheads, HD)
    wv_a = np.concatenate(
        [wv_r, np.zeros((DC, 128, NPASS, nheads, 1), wv_r.dtype)], axis=-1)
    wv_t = wv_a.transpose(2, 0, 1, 3, 4).reshape(NPASS, DC, 128, nheads * 65)
    bv_r = bv.reshape(NPASS, nheads, HD)
    bv_a = np.concatenate(
        [bv_r, np.ones((NPASS, nheads, 1), bv_r.dtype)], axis=-1)
    bv_t = bv_a.reshape(NPASS, 1, nheads * 65)

    w1_t = w1_e.reshape(DC, 128, NG, DFFm // NG).transpose(2, 0, 1, 3)
    f32c = lambda a: np.ascontiguousarray(a, dtype=np.float32)
    return dict(
        wq=f32c(qkv_tile(wq_e)), wk=f32c(qkv_tile(wk_e)),
        wv=f32c(wv_t), bv=f32c(bv_t),
        bq=f32c(bq), bk=f32c(bk),
        wo=f32c(Wo), bo=f32c(bo.reshape(1, Dm)),
        w1=f32c(w1_t), b1=f32c(b1_e),
        w2=f32c(W2), b2=f32c(np.broadcast_to(b2.reshape(1, Dm), (128, Dm))),
        ident=np.eye(128, dtype=np.float32),
        ones=np.ones((128, 1), np.float32),
        zeros=np.zeros((128, 1), np.float32),
    )


def core_plan(cfg, half):
    """Return (perm, qposA, qposB) token index arrays for one core."""
    QB = cfg["QB"]
    Tf = cfg["T"]
    nb = Tf // QB  # 4 blocks
    if half == 0:
        bA, bB = nb - 1, 0
    else:
        bA, bB = nb - 2, 1
    own = {bA, bB}
    restA = [b for b in range(nb) if b not in own and b < bA]
    restB = [b for b in range(nb) if b not in own and b >= bA]
    blocks = [bA, bB] + restA + restB
    perm = np.concatenate([np.arange(b * QB, (b + 1) * QB) for b in blocks])
    qposA = np.arange(bA * QB, (bA + 1) * QB)
    qposB = np.arange(bB * QB, (bB + 1) * QB)
    return perm, qposA, qposB


def make_masks(cfg, perm, qposA, qposB):
    """tri tiles [QBC,128,NT]; whole-chunk exp-bias scalars (0 / -80)."""
    c = derive(cfg)
    QBC, NT, S = c["QBC"], c["NT"], c["S"]
    key = perm
    tri = np.zeros((QBC, 128, NT), np.float32)
    for j in range(QBC):
        ks = key[j * 128:(j + 1) * 128]
        tri[j] = (ks[:, None] <= qposA[None, :]).astype(np.float32)
    cm = np.zeros((2 * QBC, 128, 1), np.float32)
    for j in range(QBC):
        sc = S - QBC + j
        ks = key[sc * 128:(sc + 1) * 128]
        m = ks[:, None] <= qposA[None, :]
        assert m.all() or not m.any(), "chunk not homogeneous"
        cm[j] = 0.0 if m.all() else -80.0
    for j in range(QBC):
        sc = 2 * QBC + j
        ks = key[sc * 128:(sc + 1) * 128]
        m = ks[:, None] <= qposB[None, :]
        assert m.all() or not m.any(), "chunk not homogeneous"
        cm[QBC + j] = 0.0 if m.all() else -80.0
    return tri, cm


_NC_CACHE = {}

# test-harness knobs (ignored in normal grading use)
TRACE = False
TRACE_KWARGS = {}
LAST_RESULT = None


def _get_nc(key, cfg):
    if key not in _NC_CACHE:
        _NC_CACHE[key] = build(cfg)
    return _NC_CACHE[key]


def kernel(x, Wq, Wk, Wv, Wo, bo, W1, b1, W2, b2, g1, be1, g2, be2):
    cfg = FULL_CFG
    c = derive(cfg)
    x = np.asarray(x, np.float32)
    shared = prepare_shared(cfg, np.asarray(Wq), np.asarray(Wk), np.asarray(Wv),
                            np.asarray(Wo), np.asarray(bo), np.asarray(W1),
                            np.asarray(b1), np.asarray(W2), np.asarray(b2),
                            np.asarray(g1), np.asarray(be1), np.asarray(g2),
                            np.asarray(be2))
    nc = _get_nc("full", cfg)
    in_maps = []
    plans = []
    for core in range(N_CORES):
        b, half = core // 2, core % 2
        perm, qposA, qposB = core_plan(cfg, half)
        tri, cm = make_masks(cfg, perm, qposA, qposB)
        m = dict(shared)
        m["x"] = np.ascontiguousarray(x[b][perm], np.float32)
        m["tri"] = tri
        m["cm"] = cm
        in_maps.append(m)
        plans.append((b, perm))
    res = run_bass_kernel_spmd(nc, in_maps, list(range(N_CORES)),
                               trace=TRACE, **TRACE_KWARGS)
    global LAST_RESULT
    LAST_RESULT = res
    out = np.zeros((B, T, D), np.float32)
    TOWN = c["TOWN"]
    for core in range(N_CORES):
        b, perm = plans[core]
        o = res.results[core]["out"]
        out[b][perm[:TOWN]] = o
    return out


# revision 33
# speedup vs baseline: 1.1709x; 1.1709x over previous
"""Trainium2 Bass kernel for a dense pre-LN transformer block.

Sharding: 8 cores = 4 batches x 2 sequence-halves (zigzag query blocks).
Each core handles one batch element; K/V are computed redundantly for the
full sequence on both cores of a batch (cheaper than collectives), and each
core computes attention + proj + FFN for 1024 of the 2048 query tokens.

To keep the SPMD instruction stream identical across cores, each core's
tokens are host-side permuted to [own_blockA; own_blockB; rest] and all
causal-validity variation is carried in per-core mask data (triangular
tiles for diagonal blocks, per-partition 0/1 scalars for whole chunks).

All matmuls run as float32r (FP22, full PE rate); softmax/LN in fp32.
"""

import contextlib

import numpy as np

from concourse import bass, bacc, tile, mybir
from concourse.bass_utils import run_bass_kernel_spmd

F32 = mybir.dt.float32
F32R = mybir.dt.float32r
BF16 = mybir.dt.bfloat16

B, T, D = 4, 2048, 1024
H, HD = 16, 64
DFF = 4 * D
EPS = 1e-5
N_CORES = 8

FULL_CFG = dict(D=1024, H=16, T=2048, QB=512, DFF=4096, NG=4)
SMALL_CFG = dict(D=256, H=4, T=512, QB=128, DFF=512, NG=2)


def derive(cfg):
    c = dict(cfg)
    c["DC"] = cfg["D"] // 128            # d-chunks
    c["FC"] = cfg["H"] * HD // 128       # feature chunks (head pairs)
    c["FCP"] = 2                         # f-chunks per pass
    c["NPASS"] = c["FC"] // c["FCP"]
    c["S"] = cfg["T"] // 128             # key chunks
    c["QBC"] = cfg["QB"] // 128          # chunks per query block
    c["NT"] = cfg["QB"]                  # moving-dim tile (== query block)
    c["TOWN"] = 2 * cfg["QB"]            # tokens owned per core
    c["TOC"] = c["TOWN"] // 128
    c["NO"] = min(512, cfg["D"])
    c["OC"] = cfg["D"] // c["NO"]        # dout chunks of <=512
    c["GFC"] = (cfg["DFF"] // cfg["NG"]) // 128  # f-chunks per FFN group
    c["KTB"] = cfg["T"] // c["NT"]       # t-blocks for k over full T
    return c


def build(cfg):
    """Emit the bass program for one core. Returns nc."""
    c = derive(cfg)
    Dm, Tf, DFFm, NG = cfg["D"], cfg["T"], cfg["DFF"], cfg["NG"]
    DC, FC, FCP, NPASS = c["DC"], c["FC"], c["FCP"], c["NPASS"]
    S, QBC, NT, TOWN, TOC = c["S"], c["QBC"], c["NT"], c["TOWN"], c["TOC"]
    OC, NO, GFC, KTB = c["OC"], c["NO"], c["GFC"], c["KTB"]
    HDf = HD  # 64

    nc = bacc.Bacc("TRN2", target_bir_lowering=False, debug=False)

    # ---- DRAM I/O ----
    x_d = nc.dram_tensor("x", [Tf, Dm], F32, kind="ExternalInput")
    wq_d = nc.dram_tensor("wq", [NPASS, DC, 128, FCP * 128], F32,
                          kind="ExternalInput")
    wk_d = nc.dram_tensor("wk", [NPASS, DC, 128, FCP * 128], F32,
                          kind="ExternalInput")
    wv_d = nc.dram_tensor("wv", [NPASS, DC, 128, FCP * 130], F32,
                          kind="ExternalInput")
    bq_d = nc.dram_tensor("bq", [FC, 128, 1], F32, kind="ExternalInput")
    bk_d = nc.dram_tensor("bk", [FC, 128, 1], F32, kind="ExternalInput")
    bv_d = nc.dram_tensor("bv", [NPASS, 1, FCP * 130], F32,
                          kind="ExternalInput")
    wo_d = nc.dram_tensor("wo", [FC * 128, Dm], F32, kind="ExternalInput")
    bo_d = nc.dram_tensor("bo", [1, Dm], F32, kind="ExternalInput")
    w1_d = nc.dram_tensor("w1", [NG, DC, 128, DFFm // NG], F32,
                          kind="ExternalInput")
    b1_d = nc.dram_tensor("b1", [DFFm // 128, 128, 1], F32, kind="ExternalInput")
    w2_d = nc.dram_tensor("w2", [DFFm, Dm], F32, kind="ExternalInput")
    b2_d = nc.dram_tensor("b2", [128, Dm], F32, kind="ExternalInput")
    tri_d = nc.dram_tensor("tri", [QBC, 128, NT], F32, kind="ExternalInput")
    cm_d = nc.dram_tensor("cm", [2 * QBC, 128, 1], F32, kind="ExternalInput")
    idn_d = nc.dram_tensor("ident", [128, 128], F32, kind="ExternalInput")
    one_d = nc.dram_tensor("ones", [128, 1], F32, kind="ExternalInput")
    zro_d = nc.dram_tensor("zeros", [128, 1], F32, kind="ExternalInput")
    out_d = nc.dram_tensor("out", [TOWN, Dm], F32, kind="ExternalOutput")
    x1s_d = nc.dram_tensor("x1s", [TOWN, Dm], F32)  # internal scratch

    xr = x_d.ap().rearrange("(n p) d -> n p d", p=128)
    x1r = x1s_d.ap().rearrange("(n p) d -> n p d", p=128)
    outr = out_d.ap().rearrange("(n p) d -> n p d", p=128)

    with tile.TileContext(nc) as tc, contextlib.ExitStack() as top:
        cpool = top.enter_context(tc.tile_pool(name="const", bufs=1))
        ident = cpool.tile([128, 128], F32, name="ident", tag="ident")
        nc.sync.dma_start(ident[:], idn_d.ap())
        onesc = cpool.tile([128, 1], F32R, name="onesc", tag="onesc")
        nc.sync.dma_start(onesc[:], one_d.ap().bitcast(F32R))
        cms = cpool.tile([128, 2 * QBC], F32, name="cms", tag="cms")
        for i in range(2 * QBC):
            nc.gpsimd.dma_start(cms[:, i:i + 1], cm_d.ap()[i])
        zbias = cpool.tile([128, 1], F32, name="zbias", tag="zbias")
        nc.gpsimd.dma_start(zbias[:], zro_d.ap())

        ctx_stack = contextlib.ExitStack()
        ctxp = ctx_stack.enter_context(tc.tile_pool(name="ctxTp", bufs=1))
        ctxT = [ctxp.tile([128, TOWN], F32R, name=f"ctxT{fc}", tag=f"ctxT{fc}")
                for fc in range(FC)]

        hT_stack = contextlib.ExitStack()
        hp = hT_stack.enter_context(tc.tile_pool(name="hTp", bufs=1))
        hT = [hp.tile([128, Tf], F32R, name=f"hT{dc}", tag=f"hT{dc}")
              for dc in range(DC)]
        trip = hT_stack.enter_context(tc.tile_pool(name="trip", bufs=1))
        tri = []
        for i in range(QBC):
            m = trip.tile([128, NT], F32R, name=f"tri{i}", tag=f"tri{i}")
            nc.gpsimd.dma_start(m[:], tri_d.ap()[i].bitcast(F32R))
            tri.append(m)

        # ---------------- Phase 1: LN1 + transpose -> hT ----------------
        with tc.tile_pool(name="ln1", bufs=4) as lp, \
             tc.tile_pool(name="ln1s", bufs=8) as lsp, \
             tc.tile_pool(name="ln1p", bufs=4, space=bass.MemorySpace.PSUM) as lpp:
            for ti in range(S):
                xt = lp.tile([128, Dm], F32, name="xt", tag="xt")
                nc.sync.dma_start(xt[:], xr[ti])
                nsub = max(1, Dm // 512)
                st6 = lsp.tile([128, nsub, 6], F32, name="st6", tag="st6")
                for sb_i in range(nsub):
                    nc.vector.bn_stats(
                        st6[:, sb_i, :],
                        xt[:, sb_i * (Dm // nsub):(sb_i + 1) * (Dm // nsub)])
                agg = lsp.tile([128, 2], F32, name="agg", tag="agg")
                nc.vector.bn_aggr(agg[:], st6[:])
                veps = lsp.tile([128, 1], F32, name="veps",
                                  tag="veps")
                nc.vector.tensor_scalar_add(veps[:], agg[:, 1:2], EPS)
                std = lsp.tile([128, 1], F32, name="std", tag="std")
                nc.scalar.sqrt(std[:], veps[:])
                rstd = lsp.tile([128, 1], F32, name="rstd", tag="rstd")
                nc.vector.reciprocal(rstd[:], std[:])
                ht = lp.tile([128, Dm], F32, name="ht", tag="ht")
                nc.vector.tensor_scalar(ht[:], xt[:], agg[:, 0:1], rstd[:],
                                        op0=mybir.AluOpType.subtract,
                                        op1=mybir.AluOpType.mult)
                for dc in range(DC):
                    ps = lpp.tile([128, 128], F32, name="tps", tag="tps")
                    nc.tensor.transpose(ps[:], ht[:, dc * 128:(dc + 1) * 128],
                                        ident[:])
                    if dc % 2 == 0:
                        nc.scalar.copy(
                            hT[dc][:, ti * 128:(ti + 1) * 128], ps[:])
                    else:
                        nc.vector.tensor_copy(
                            hT[dc][:, ti * 128:(ti + 1) * 128], ps[:])

        # ---------------- Phase 2: per-pass QKV + attention -------------
        with tc.tile_pool(name="pass_sb", bufs=1) as pp, \
             tc.tile_pool(name="vtp", bufs=1) as vp, \
             tc.tile_pool(name="wvres", bufs=1) as wvp, \
             tc.tile_pool(name="expp", bufs=4) as ep, \
             tc.tile_pool(name="zrowp", bufs=2) as zp, \
             tc.tile_pool(name="zbp", bufs=2) as zbp, \
             tc.tile_pool(name="qkvps", bufs=2, space=bass.MemorySpace.PSUM) as qps, \
             tc.tile_pool(name="scps", bufs=2, space=bass.MemorySpace.PSUM) as sps, \
             tc.tile_pool(name="ctxps", bufs=1, space=bass.MemorySpace.PSUM) as cps:
            for p in range(NPASS):
                fcs = [p * FCP + i for i in range(FCP)]
                # --- K^T and Q^T (feature-major) ---
                kT = [pp.tile([128, Tf], F32R, name=f"kT{i}", tag=f"kT{i}")
                      for i in range(FCP)]
                qT = [pp.tile([128, 2 * NT], F32R, name=f"qT{i}", tag=f"qT{i}")
                      for i in range(FCP)]
                bks = []
                bqs = []
                for i, fc in enumerate(fcs):
                    bkt = pp.tile([128, 1], F32, name=f"bk{i}", tag=f"bk{i}")
                    nc.sync.dma_start(bkt[:], bk_d.ap()[fc])
                    bks.append(bkt)
                    bqt = pp.tile([128, 1], F32, name=f"bq{i}", tag=f"bq{i}")
                    nc.sync.dma_start(bqt[:], bq_d.ap()[fc])
                    bqs.append(bqt)
                wkp = []
                wqp = []
                for dc in range(DC):
                    wkt = pp.tile([128, FCP * 128], F32R,
                                  name=f"wkp{dc}", tag=f"wkp{dc}")
                    nc.gpsimd.dma_start(wkt[:], wk_d.ap()[p, dc].bitcast(F32R))
                    wkp.append(wkt)
                    wqt = pp.tile([128, FCP * 128], F32R,
                                  name=f"wqp{dc}", tag=f"wqp{dc}")
                    nc.gpsimd.dma_start(wqt[:], wq_d.ap()[p, dc].bitcast(F32R))
                    wqp.append(wqt)
                for i, fc in enumerate(fcs):
                    ws = slice(i * 128, (i + 1) * 128)
                    for tb in range(KTB):
                        pk = qps.tile([128, NT], F32, name="pk", tag="qkv")
                        for dc in range(DC):
                            nc.tensor.matmul(
                                pk[:], (wkp[dc][:, ws]),
                                (hT[dc][:, tb * NT:(tb + 1) * NT]),
                                start=(dc == 0), stop=(dc == DC - 1))
                        nc.vector.tensor_scalar_add(
                            kT[i][:, tb * NT:(tb + 1) * NT], pk[:], bks[i][:])
                    for tb in range(2):
                        pq = qps.tile([128, NT], F32, name="pq", tag="qkv")
                        for dc in range(DC):
                            nc.tensor.matmul(
                                pq[:], (wqp[dc][:, ws]),
                                (hT[dc][:, tb * NT:(tb + 1) * NT]),
                                start=(dc == 0), stop=(dc == DC - 1))
                        nc.vector.tensor_scalar_add(
                            qT[i][:, tb * NT:(tb + 1) * NT], pq[:], bqs[i][:])
                # --- V (token-major) ---
                bvr = pp.tile([1, FCP * 130], F32, name="bvr", tag="bvr")
                nc.sync.dma_start(bvr[:], bv_d.ap()[p])
                bvb = pp.tile([128, FCP * 130], F32, name="bvb", tag="bvb")
                nc.gpsimd.partition_broadcast(bvb[:], bvr[:])
                wvs = []
                for dc in range(DC):
                    wvt = wvp.tile([128, FCP * 130], F32R,
                                   name=f"wv{dc}", tag=f"wv{dc}")
                    nc.gpsimd.dma_start(wvt[:], wv_d.ap()[p, dc].bitcast(F32R))
                    wvs.append(wvt)
                vt = [vp.tile([128, FCP * 130], F32R, name=f"v{ti}",
                              tag=f"v{ti}") for ti in range(S)]
                for ti in range(S):
                    pv = qps.tile([128, FCP * 130], F32, name="pv", tag="qkv")
                    for dc in range(DC):
                        nc.tensor.matmul(
                            pv[:], (hT[dc][:, ti * 128:(ti + 1) * 128]),
                            (wvs[dc][:]),
                            start=(dc == 0), stop=(dc == DC - 1))
                    nc.vector.tensor_add(vt[ti][:], pv[:], bvb[:])
                # --- attention per head pair ---
                for i, fc in enumerate(fcs):
                    for qb in range(2):
                        if qb == 0:
                            schunks = list(range(S))
                        else:
                            schunks = list(range(QBC, 3 * QBC))
                        ctx_ps = [cps.tile([65, NT], F32, name=f"ctx{hh}",
                                           tag=f"ctx{hh}") for hh in range(2)]
                        nsc = len(schunks)
                        for idx, sc in enumerate(schunks):
                            # mask: (kind, index); kind: 0=none,1=tri,2=scalar
                            if qb == 0:
                                if sc < QBC:
                                    mk = (1, sc)
                                elif sc >= S - QBC:
                                    mk = (2, sc - (S - QBC))
                                else:
                                    mk = (0, 0)
                            else:
                                if sc < 2 * QBC:
                                    mk = (1, sc - QBC)
                                else:
                                    mk = (2, QBC + (sc - 2 * QBC))
                            sps_t = sps.tile([128, 2, NT], F32,
                                             name="sc", tag="sc")
                            e2 = ep.tile([128, 2, NT], F32R, name="e", tag="e")
                            for hh in range(2):
                                rows = slice(hh * HDf, (hh + 1) * HDf)
                                nc.tensor.matmul(
                                    sps_t[:, hh, :],
                                    (kT[i][rows, sc * 128:(sc + 1) * 128]),
                                    (qT[i][rows, qb * NT:(qb + 1) * NT]),
                                    start=True, stop=True,
                                    tile_position=(hh * HDf, 0))
                            ebias = cms[:, mk[1]:mk[1] + 1] \
                                if mk[0] == 2 else zbias[:]
                            nc.scalar.activation(
                                e2[:], sps_t[:],
                                mybir.ActivationFunctionType.Exp,
                                bias=ebias)
                            if mk[0] == 1:
                                nc.vector.tensor_mul(
                                    e2[:], e2[:],
                                    tri[mk[1]][:].unsqueeze(1)
                                    .to_broadcast([128, 2, NT]))
                            for hh in range(2):
                                nc.tensor.matmul(
                                    ctx_ps[hh][:],
                                    (vt[sc][:, (i * 2 + hh) * 65:
                                             (i * 2 + hh) * 65 + 65]),
                                    (e2[:, hh, :]),
                                    start=(idx == 0), stop=(idx == nsc - 1),
                                    skip_group_check=True)
                        for hh in range(2):
                            zrow = zp.tile([1, NT], F32, name="zrow",
                                           tag="zrow")
                            nc.vector.tensor_copy(zrow[:], ctx_ps[hh][64:65, :])
                            rz = zp.tile([1, NT], F32, name="rz", tag="rz")
                            nc.vector.reciprocal(rz[:], zrow[:])
                            zb = zbp.tile([64, NT], F32, name="zb", tag="zb")
                            nc.gpsimd.partition_broadcast(zb[:], rz[:])
                            rows = slice(hh * HDf, (hh + 1) * HDf)
                            nc.vector.tensor_mul(
                                ctxT[fc][rows, qb * NT:(qb + 1) * NT],
                                ctx_ps[hh][0:64, :], zb[:])

        hT_stack.close()

        # ---------------- Phase 3: projection + fused LN2 ---------------
        h2_stack = contextlib.ExitStack()
        h2p = h2_stack.enter_context(tc.tile_pool(name="h2Tp", bufs=1))
        h2T = [h2p.tile([128, TOWN], F32R, name=f"h2T{dc}", tag=f"h2T{dc}")
               for dc in range(DC)]
        with tc.tile_pool(name="proj_sb", bufs=1) as prp, \
             tc.tile_pool(name="proj_x", bufs=3) as pxp, \
             tc.tile_pool(name="proj_o", bufs=4) as pop, \
             tc.tile_pool(name="ln2s", bufs=8) as lsp2, \
             tc.tile_pool(name="ln2h", bufs=4) as lph2, \
             tc.tile_pool(name="projps", bufs=3, space=bass.MemorySpace.PSUM) as pps, \
             tc.tile_pool(name="ln2p", bufs=4, space=bass.MemorySpace.PSUM) as lpp2:
            bo_row = prp.tile([1, Dm], F32, name="bo_row", tag="bo_row")
            nc.sync.dma_start(bo_row[:], bo_d.ap())
            bob = prp.tile([128, Dm], F32, name="bob", tag="bob")
            nc.gpsimd.partition_broadcast(bob[:], bo_row[:])

            wos = []
            for fc in range(FC):
                wot = prp.tile([128, Dm], F32R, name=f"wo{fc}", tag=f"wo{fc}")
                nc.gpsimd.dma_start(
                    wot[:], wo_d.ap()[fc * 128:(fc + 1) * 128, :].bitcast(F32R))
                wos.append(wot)
            for ti in range(TOC):
                xo = pxp.tile([128, Dm], F32, name="xo", tag="xo")
                nc.sync.dma_start(xo[:], xr[ti])
                x1t = pop.tile([128, Dm], F32, name="x1t", tag="x1t")
                for oc in range(OC):
                    ppt = pps.tile([128, NO], F32, name="ppt", tag="ppt")
                    for fc in range(FC):
                        nc.tensor.matmul(
                            ppt[:],
                            (ctxT[fc][:, ti * 128:(ti + 1) * 128]),
                            (wos[fc][:, oc * NO:(oc + 1) * NO]),
                            start=(fc == 0), stop=(fc == FC - 1))
                    cols = slice(oc * NO, (oc + 1) * NO)
                    nc.vector.tensor_add(x1t[:, cols], ppt[:], xo[:, cols])
                    nc.vector.tensor_add(x1t[:, cols], x1t[:, cols],
                                         bob[:, cols])
                # fused LN2 on the freshly built x1 tile
                nsub = max(1, Dm // 512)
                st6 = lsp2.tile([128, nsub, 6], F32, name="st6b", tag="st6b")
                for sb_i in range(nsub):
                    nc.vector.bn_stats(
                        st6[:, sb_i, :],
                        x1t[:, sb_i * (Dm // nsub):(sb_i + 1) * (Dm // nsub)])
                agg = lsp2.tile([128, 2], F32, name="aggb", tag="aggb")
                nc.vector.bn_aggr(agg[:], st6[:])
                veps = lsp2.tile([128, 1], F32, name="vepsb", tag="vepsb")
                nc.vector.tensor_scalar_add(veps[:], agg[:, 1:2], EPS)
                std = lsp2.tile([128, 1], F32, name="stdb", tag="stdb")
                nc.scalar.sqrt(std[:], veps[:])
                rstd = lsp2.tile([128, 1], F32, name="rstdb", tag="rstdb")
                nc.vector.reciprocal(rstd[:], std[:])
                hb = lph2.tile([128, Dm], F32, name="hb", tag="hb")
                nc.vector.tensor_scalar(hb[:], x1t[:], agg[:, 0:1], rstd[:],
                                        op0=mybir.AluOpType.subtract,
                                        op1=mybir.AluOpType.mult)
                for dc in range(DC):
                    ps2 = lpp2.tile([128, 128], F32, name="tps2", tag="tps2")
                    nc.tensor.transpose(ps2[:], hb[:, dc * 128:(dc + 1) * 128],
                                        ident[:])
                    nc.scalar.copy(
                        h2T[dc][:, ti * 128:(ti + 1) * 128], ps2[:])
                nc.sync.dma_start(x1r[ti], x1t[:])

        # (LN2 is fused into the projection loop above; h2T ready here.)
        if False:
            for ti in range(TOC):
                nsub = max(1, Dm // 512)
                pass

        # ---------------- Phase 5: FFN ----------------------------------
        with tc.tile_pool(name="ffn_sb", bufs=1) as fp, \
             tc.tile_pool(name="ffn_w1", bufs=1) as w1p, \
             tc.tile_pool(name="ffn_w2", bufs=8) as w2p, \
             tc.tile_pool(name="ffn_b1", bufs=4) as b1p, \
             tc.tile_pool(name="ffn_x1", bufs=1) as fxp, \
             tc.tile_pool(name="ffn_out", bufs=2) as fop, \
             tc.tile_pool(name="ffps", bufs=3, space=bass.MemorySpace.PSUM) as fps, \
             tc.tile_pool(name="outps", bufs=3, space=bass.MemorySpace.PSUM) as ops:
            b2b = fp.tile([128, Dm], F32, name="b2b", tag="b2b")
            nc.sync.dma_start(b2b[:], b2_d.ap())
            oacc = [fp.tile([128, Dm], F32, name=f"oacc{ti}", tag=f"oacc{ti}")
                    for ti in range(TOC)]
            ffT = [fp.tile([128, TOWN], F32R, name=f"ffT{j}", tag=f"ffT{j}")
                   for j in range(GFC)]
            for g in range(NG):
                w1g = []
                for dc in range(DC):
                    w1t = w1p.tile([128, DFFm // NG], F32R,
                                   name=f"w1g{dc}", tag=f"w1g{dc}")
                    nc.gpsimd.dma_start(w1t[:], w1_d.ap()[g, dc].bitcast(F32R))
                    w1g.append(w1t)
                for j in range(GFC):
                    gf = g * GFC + j
                    b1t = b1p.tile([128, 1], F32, name="b1t", tag="b1t")
                    nc.gpsimd.dma_start(b1t[:], b1_d.ap()[gf])
                    for tb in range(TOWN // NT):
                        fpt = fps.tile([128, NT], F32, name="fpt", tag="fpt")
                        for dc in range(DC):
                            nc.tensor.matmul(
                                fpt[:], (w1g[dc][:, j * 128:(j + 1) * 128]),
                                (h2T[dc][:, tb * NT:(tb + 1) * NT]),
                                start=(dc == 0), stop=(dc == DC - 1))
                        nc.scalar.activation(
                            ffT[j][:, tb * NT:(tb + 1) * NT], fpt[:],
                            mybir.ActivationFunctionType.Relu,
                            bias=b1t[:])
                w2s = []
                for j in range(GFC):
                    gf = g * GFC + j
                    w2t = w2p.tile([128, Dm], F32R, name="w2t", tag="w2t")
                    nc.gpsimd.dma_start(
                        w2t[:], w2_d.ap()[gf * 128:(gf + 1) * 128, :]
                        .bitcast(F32R))
                    w2s.append(w2t)
                for ti in range(TOC):
                    x1t = None
                    if g == NG - 1:
                        x1t = fxp.tile([128, Dm], F32, name="x1f", tag="x1f")
                        nc.sync.dma_start(x1t[:], x1r[ti])
                    for oc in range(OC):
                        opt = ops.tile([128, NO], F32, name="opt", tag="opt")
                        for j in range(GFC):
                            nc.tensor.matmul(
                                opt[:],
                                (ffT[j][:, ti * 128:(ti + 1) * 128]),
                                (w2s[j][:, oc * NO:(oc + 1) * NO]),
                                start=(j == 0), stop=(j == GFC - 1))
                        cols = slice(oc * NO, (oc + 1) * NO)
                        if g == 0:
                            nc.vector.tensor_copy(oacc[ti][:, cols], opt[:])
                        elif g < NG - 1:
                            nc.vector.tensor_add(oacc[ti][:, cols],
                                                 oacc[ti][:, cols], opt[:])
                        else:
                            nc.vector.tensor_add(oacc[ti][:, cols],
                                                 oacc[ti][:, cols], opt[:])
                            nc.vector.tensor_add(oacc[ti][:, cols],
                                                 oacc[ti][:, cols],
                                                 x1t[:, cols])
                            ot = fop.tile([128, NO], F32, name="ot", tag="ot")
                            nc.vector.tensor_add(ot[:], oacc[ti][:, cols],
                                                 b2b[:, cols])
                            nc.sync.dma_start(outr[ti][:, cols], ot[:])
        h2_stack.close()
        ctx_stack.close()
    nc.compile()
    return nc


# ---------------------------------------------------------------------------
# host-side input preparation
# ---------------------------------------------------------------------------

def prepare_shared(cfg, Wq, Wk, Wv, Wo, bo, W1, b1, W2, b2, g1, be1, g2, be2):
    c = derive(cfg)
    Dm, Hn, DFFm, FC = cfg["D"], cfg["H"], cfg["DFF"], c["FC"]
    scale = 1.0 / np.sqrt(HD)
    wq_f = np.ascontiguousarray(Wq.transpose(1, 0, 2).reshape(Dm, Hn * HD))
    wk_f = np.ascontiguousarray(Wk.transpose(1, 0, 2).reshape(Dm, Hn * HD))
    wv_f = np.ascontiguousarray(Wv.transpose(1, 0, 2).reshape(Dm, Hn * HD))
    wq_e = (g1[:, None] * wq_f) * scale
    wk_e = g1[:, None] * wk_f
    wv_e = g1[:, None] * wv_f
    bq = ((be1 @ wq_f) * scale).reshape(FC, 128, 1)
    bk = (be1 @ wk_f).reshape(FC, 128, 1)
    bv = (be1 @ wv_f).reshape(1, Hn * HD)
    w1_e = g2[:, None] * W1
    b1_e = (b1 + be2 @ W1).reshape(DFFm // 128, 128, 1)
    DC, NPASS, FCP, NG = c["DC"], c["NPASS"], c["FCP"], cfg["NG"]

    def qkv_tile(w):
        # [D, F] -> [NPASS, DC, 128, FCP*128]
        return w.reshape(DC, 128, NPASS, FCP * 128).transpose(2, 0, 1, 3)

    # v weights get a zero column appended per head; its bias is 1.0, so the
    # v tiles come out of the matmul+bias with a built-in ones column that
    # accumulates the softmax normalizer during the ctx matmul.
    nheads = FCP * 2
    wv_r = wv_e.reshape(DC, 128, NPASS, nheads, HD)
    wv_a = np.concatenate(
        [wv_r, np.zeros((DC, 128, NPASS, nheads, 1), wv_r.dtype)], axis=-1)
    wv_t = wv_a.transpose(2, 0, 1, 3, 4).reshape(NPASS, DC, 128, nheads * 65)
    bv_r = bv.reshape(NPASS, nheads, HD)
    bv_a = np.concatenate(
        [bv_r, np.ones((NPASS, nheads, 1), bv_r.dtype)], axis=-1)
    bv_t = bv_a.reshape(NPASS, 1, nheads * 65)

    w1_t = w1_e.reshape(DC, 128, NG, DFFm // NG).transpose(2, 0, 1, 3)
    f32c = lambda a: np.ascontiguousarray(a, dtype=np.float32)
    return dict(
        wq=f32c(qkv_tile(wq_e)), wk=f32c(qkv_tile(wk_e)),
        wv=f32c(wv_t), bv=f32c(bv_t),
        bq=f32c(bq), bk=f32c(bk),
        wo=f32c(Wo), bo=f32c(bo.reshape(1, Dm)),
        w1=f32c(w1_t), b1=f32c(b1_e),
        w2=f32c(W2), b2=f32c(np.broadcast_to(b2.reshape(1, Dm), (128, Dm))),
        ident=np.eye(128, dtype=np.float32),
        ones=np.ones((128, 1), np.float32),
        zeros=np.zeros((128, 1), np.float32),
    )


def core_plan(cfg, half):
    """Return (perm, qposA, qposB) token index arrays for one core."""
    QB = cfg["QB"]
    Tf = cfg["T"]
    nb = Tf // QB  # 4 blocks
    if half == 0:
        bA, bB = nb - 1, 0
    else:
        bA, bB = nb - 2, 1
    own = {bA, bB}
    restA = [b for b in range(nb) if b not in own and b < bA]
    restB = [b for b in range(nb) if b not in own and b >= bA]
    blocks = [bA, bB] + restA + restB
    perm = np.concatenate([np.arange(b * QB, (b + 1) * QB) for b in blocks])
    qposA = np.arange(bA * QB, (bA + 1) * QB)
    qposB = np.arange(bB * QB, (bB + 1) * QB)
    return perm, qposA, qposB


def make_masks(cfg, perm, qposA, qposB):
    """tri tiles [QBC,128,NT]; whole-chunk exp-bias scalars (0 / -80)."""
    c = derive(cfg)
    QBC, NT, S = c["QBC"], c["NT"], c["S"]
    key = perm
    tri = np.zeros((QBC, 128, NT), np.float32)
    for j in range(QBC):
        ks = key[j * 128:(j + 1) * 128]
        tri[j] = (ks[:, None] <= qposA[None, :]).astype(np.float32)
    cm = np.zeros((2 * QBC, 128, 1), np.float32)
    for j in range(QBC):
        sc = S - QBC + j
        ks = key[sc * 128:(sc + 1) * 128]
        m = ks[:, None] <= qposA[None, :]
        assert m.all() or not m.any(), "chunk not homogeneous"
        cm[j] = 0.0 if m.all() else -80.0
    for j in range(QBC):
        sc = 2 * QBC + j
        ks = key[sc * 128:(sc + 1) * 128]
        m = ks[:, None] <= qposB[None, :]
        assert m.all() or not m.any(), "chunk not homogeneous"
        cm[QBC + j] = 0.0 if m.all() else -80.0
    return tri, cm


_NC_CACHE = {}

# test-harness knobs (ignored in normal grading use)
TRACE = False
TRACE_KWARGS = {}
LAST_RESULT = None


def _get_nc(key, cfg):
    if key not in _NC_CACHE:
        _NC_CACHE[key] = build(cfg)
    return _NC_CACHE[key]


def kernel(x, Wq, Wk, Wv, Wo, bo, W1, b1, W2, b2, g1, be1, g2, be2):
    cfg = FULL_CFG
    c = derive(cfg)
    x = np.asarray(x, np.float32)
    shared = prepare_shared(cfg, np.asarray(Wq), np.asarray(Wk), np.asarray(Wv),
                            np.asarray(Wo), np.asarray(bo), np.asarray(W1),
                            np.asarray(b1), np.asarray(W2), np.asarray(b2),
                            np.asarray(g1), np.asarray(be1), np.asarray(g2),
                            np.asarray(be2))
    nc = _get_nc("full", cfg)
    in_maps = []
    plans = []
    for core in range(N_CORES):
        b, half = core // 2, core % 2
        perm, qposA, qposB = core_plan(cfg, half)
        tri, cm = make_masks(cfg, perm, qposA, qposB)
        m = dict(shared)
        m["x"] = np.ascontiguousarray(x[b][perm], np.float32)
        m["tri"] = tri
        m["cm"] = cm
        in_maps.append(m)
        plans.append((b, perm))
    res = run_bass_kernel_spmd(nc, in_maps, list(range(N_CORES)),
                               trace=TRACE, **TRACE_KWARGS)
    global LAST_RESULT
    LAST_RESULT = res
    out = np.zeros((B, T, D), np.float32)
    TOWN = c["TOWN"]
    for core in range(N_CORES):
        b, perm = plans[core]
        o = res.results[core]["out"]
        out[b][perm[:TOWN]] = o
    return out


# revision 36
# speedup vs baseline: 1.2373x; 1.0568x over previous
"""Trainium2 Bass kernel for a dense pre-LN transformer block.

Sharding: 8 cores = 4 batches x 2 sequence-halves (zigzag query blocks).
Each core handles one batch element; K/V are computed redundantly for the
full sequence on both cores of a batch (cheaper than collectives), and each
core computes attention + proj + FFN for 1024 of the 2048 query tokens.

To keep the SPMD instruction stream identical across cores, each core's
tokens are host-side permuted to [own_blockA; own_blockB; rest] and all
causal-validity variation is carried in per-core mask data (triangular
tiles for diagonal blocks, per-partition 0/1 scalars for whole chunks).

All matmuls run as float32r (FP22, full PE rate); softmax/LN in fp32.
"""

import contextlib

import numpy as np

from concourse import bass, bacc, tile, mybir
from concourse.bass_utils import run_bass_kernel_spmd

F32 = mybir.dt.float32
F32R = mybir.dt.float32r
BF16 = mybir.dt.bfloat16

B, T, D = 4, 2048, 1024
H, HD = 16, 64
DFF = 4 * D
EPS = 1e-5
N_CORES = 8

FULL_CFG = dict(D=1024, H=16, T=2048, QB=512, DFF=4096, NG=4)
SMALL_CFG = dict(D=256, H=4, T=512, QB=128, DFF=512, NG=2)


def derive(cfg):
    c = dict(cfg)
    c["DC"] = cfg["D"] // 128            # d-chunks
    c["FC"] = cfg["H"] * HD // 128       # feature chunks (head pairs)
    c["FCP"] = 2                         # f-chunks per pass
    c["NPASS"] = c["FC"] // c["FCP"]
    c["S"] = cfg["T"] // 128             # key chunks
    c["QBC"] = cfg["QB"] // 128          # chunks per query block
    c["NT"] = cfg["QB"]                  # moving-dim tile (== query block)
    c["TOWN"] = 2 * cfg["QB"]            # tokens owned per core
    c["TOC"] = c["TOWN"] // 128
    c["NO"] = min(512, cfg["D"])
    c["OC"] = cfg["D"] // c["NO"]        # dout chunks of <=512
    c["GFC"] = (cfg["DFF"] // cfg["NG"]) // 128  # f-chunks per FFN group
    c["KTB"] = cfg["T"] // c["NT"]       # t-blocks for k over full T
    return c


def build(cfg):
    """Emit the bass program for one core. Returns nc."""
    c = derive(cfg)
    Dm, Tf, DFFm, NG = cfg["D"], cfg["T"], cfg["DFF"], cfg["NG"]
    DC, FC, FCP, NPASS = c["DC"], c["FC"], c["FCP"], c["NPASS"]
    S, QBC, NT, TOWN, TOC = c["S"], c["QBC"], c["NT"], c["TOWN"], c["TOC"]
    OC, NO, GFC, KTB = c["OC"], c["NO"], c["GFC"], c["KTB"]
    HDf = HD  # 64

    nc = bacc.Bacc("TRN2", target_bir_lowering=False, debug=False)

    # ---- DRAM I/O ----
    x_d = nc.dram_tensor("x", [Tf, Dm], F32, kind="ExternalInput")
    wq_d = nc.dram_tensor("wq", [NPASS, DC, 128, FCP * 128], F32,
                          kind="ExternalInput")
    wk_d = nc.dram_tensor("wk", [NPASS, DC, 128, FCP * 128], F32,
                          kind="ExternalInput")
    wv_d = nc.dram_tensor("wv", [NPASS, DC, 128, FCP * 130], F32,
                          kind="ExternalInput")
    bq_d = nc.dram_tensor("bq", [FC, 128, 1], F32, kind="ExternalInput")
    bk_d = nc.dram_tensor("bk", [FC, 128, 1], F32, kind="ExternalInput")
    bv_d = nc.dram_tensor("bv", [NPASS, 1, FCP * 130], F32,
                          kind="ExternalInput")
    wo_d = nc.dram_tensor("wo", [FC * 128, Dm], F32, kind="ExternalInput")
    bo_d = nc.dram_tensor("bo", [1, Dm], F32, kind="ExternalInput")
    w1_d = nc.dram_tensor("w1", [NG, DC, 128, DFFm // NG], F32,
                          kind="ExternalInput")
    b1_d = nc.dram_tensor("b1", [DFFm // 128, 128, 1], F32, kind="ExternalInput")
    w2_d = nc.dram_tensor("w2", [DFFm, Dm], F32, kind="ExternalInput")
    b2_d = nc.dram_tensor("b2", [128, Dm], F32, kind="ExternalInput")
    tri_d = nc.dram_tensor("tri", [QBC, 128, NT], F32, kind="ExternalInput")
    cm_d = nc.dram_tensor("cm", [2 * QBC, 128, 1], F32, kind="ExternalInput")
    idn_d = nc.dram_tensor("ident", [128, 128], F32, kind="ExternalInput")
    one_d = nc.dram_tensor("ones", [128, 1], F32, kind="ExternalInput")
    zro_d = nc.dram_tensor("zeros", [128, 1], F32, kind="ExternalInput")
    out_d = nc.dram_tensor("out", [TOWN, Dm], F32, kind="ExternalOutput")
    x1s_d = nc.dram_tensor("x1s", [TOWN, Dm], F32)  # internal scratch

    xr = x_d.ap().rearrange("(n p) d -> n p d", p=128)
    x1r = x1s_d.ap().rearrange("(n p) d -> n p d", p=128)
    outr = out_d.ap().rearrange("(n p) d -> n p d", p=128)

    with tile.TileContext(nc) as tc, contextlib.ExitStack() as top:
        cpool = top.enter_context(tc.tile_pool(name="const", bufs=1))
        ident = cpool.tile([128, 128], F32, name="ident", tag="ident")
        nc.sync.dma_start(ident[:], idn_d.ap())
        onesc = cpool.tile([128, 1], F32R, name="onesc", tag="onesc")
        nc.sync.dma_start(onesc[:], one_d.ap().bitcast(F32R))
        cms = cpool.tile([128, 2 * QBC], F32, name="cms", tag="cms")
        for i in range(2 * QBC):
            nc.gpsimd.dma_start(cms[:, i:i + 1], cm_d.ap()[i])
        zbias = cpool.tile([128, 1], F32, name="zbias", tag="zbias")
        nc.gpsimd.dma_start(zbias[:], zro_d.ap())

        ctx_stack = contextlib.ExitStack()
        ctxp = ctx_stack.enter_context(tc.tile_pool(name="ctxTp", bufs=1))
        ctxT = [ctxp.tile([128, TOWN], F32R, name=f"ctxT{fc}", tag=f"ctxT{fc}")
                for fc in range(FC)]

        hT_stack = contextlib.ExitStack()
        hp = hT_stack.enter_context(tc.tile_pool(name="hTp", bufs=1))
        hT = [hp.tile([128, Tf], F32R, name=f"hT{dc}", tag=f"hT{dc}")
              for dc in range(DC)]
        trip = hT_stack.enter_context(tc.tile_pool(name="trip", bufs=1))
        tri = []
        for i in range(QBC):
            m = trip.tile([128, NT], F32R, name=f"tri{i}", tag=f"tri{i}")
            nc.gpsimd.dma_start(m[:], tri_d.ap()[i].bitcast(F32R))
            tri.append(m)

        # ---------------- Phase 1: LN1 + transpose -> hT ----------------
        with tc.tile_pool(name="ln1", bufs=4) as lp, \
             tc.tile_pool(name="ln1s", bufs=8) as lsp, \
             tc.tile_pool(name="ln1p", bufs=4, space=bass.MemorySpace.PSUM) as lpp:
            for ti in range(S):
                xt = lp.tile([128, Dm], F32, name="xt", tag="xt")
                nsub = max(1, Dm // 512)
                st6 = lsp.tile([128, nsub, 6], F32, name="st6", tag="st6")
                for sb_i in range(nsub):
                    cs = slice(sb_i * (Dm // nsub), (sb_i + 1) * (Dm // nsub))
                    nc.sync.dma_start(xt[:, cs], xr[ti][:, cs])
                    nc.vector.bn_stats(st6[:, sb_i, :], xt[:, cs])
                agg = lsp.tile([128, 2], F32, name="agg", tag="agg")
                nc.vector.bn_aggr(agg[:], st6[:])
                veps = lsp.tile([128, 1], F32, name="veps",
                                  tag="veps")
                nc.vector.tensor_scalar_add(veps[:], agg[:, 1:2], EPS)
                std = lsp.tile([128, 1], F32, name="std", tag="std")
                nc.scalar.sqrt(std[:], veps[:])
                rstd = lsp.tile([128, 1], F32, name="rstd", tag="rstd")
                nc.vector.reciprocal(rstd[:], std[:])
                ht = lp.tile([128, Dm], F32, name="ht", tag="ht")
                hstep = Dm // nsub
                for sb_i in range(nsub):
                    cs = slice(sb_i * hstep, (sb_i + 1) * hstep)
                    nc.vector.tensor_scalar(ht[:, cs], xt[:, cs],
                                            agg[:, 0:1], rstd[:],
                                            op0=mybir.AluOpType.subtract,
                                            op1=mybir.AluOpType.mult)
                    for dc in range(sb_i * DC // nsub,
                                    (sb_i + 1) * DC // nsub):
                        ps = lpp.tile([128, 128], F32, name="tps", tag="tps")
                        nc.tensor.transpose(
                            ps[:], ht[:, dc * 128:(dc + 1) * 128], ident[:])
                        if dc % 2 == 0:
                            nc.scalar.copy(
                                hT[dc][:, ti * 128:(ti + 1) * 128], ps[:])
                        else:
                            nc.vector.tensor_copy(
                                hT[dc][:, ti * 128:(ti + 1) * 128], ps[:])

        # ---------------- Phase 2: per-pass QKV + attention -------------
        with tc.tile_pool(name="pass_sb", bufs=1) as pp, \
             tc.tile_pool(name="vtp", bufs=1) as vp, \
             tc.tile_pool(name="wvres", bufs=1) as wvp, \
             tc.tile_pool(name="expp", bufs=4) as ep, \
             tc.tile_pool(name="zrowp", bufs=2) as zp, \
             tc.tile_pool(name="zbp", bufs=2) as zbp, \
             tc.tile_pool(name="qkvps", bufs=2, space=bass.MemorySpace.PSUM) as qps, \
             tc.tile_pool(name="scps", bufs=2, space=bass.MemorySpace.PSUM) as sps, \
             tc.tile_pool(name="ctxps", bufs=1, space=bass.MemorySpace.PSUM) as cps:
            for p in range(NPASS):
                fcs = [p * FCP + i for i in range(FCP)]
                # --- K^T and Q^T (feature-major) ---
                kT = [pp.tile([128, Tf], F32R, name=f"kT{i}", tag=f"kT{i}")
                      for i in range(FCP)]
                qT = [pp.tile([128, 2 * NT], F32R, name=f"qT{i}", tag=f"qT{i}")
                      for i in range(FCP)]
                bks = []
                bqs = []
                for i, fc in enumerate(fcs):
                    bkt = pp.tile([128, 1], F32, name=f"bk{i}", tag=f"bk{i}")
                    nc.sync.dma_start(bkt[:], bk_d.ap()[fc])
                    bks.append(bkt)
                    bqt = pp.tile([128, 1], F32, name=f"bq{i}", tag=f"bq{i}")
                    nc.sync.dma_start(bqt[:], bq_d.ap()[fc])
                    bqs.append(bqt)
                wkp = []
                wqp = []
                for dc in range(DC):
                    wkt = pp.tile([128, FCP * 128], F32R,
                                  name=f"wkp{dc}", tag=f"wkp{dc}")
                    nc.gpsimd.dma_start(wkt[:], wk_d.ap()[p, dc].bitcast(F32R))
                    wkp.append(wkt)
                    wqt = pp.tile([128, FCP * 128], F32R,
                                  name=f"wqp{dc}", tag=f"wqp{dc}")
                    nc.gpsimd.dma_start(wqt[:], wq_d.ap()[p, dc].bitcast(F32R))
                    wqp.append(wqt)
                for i, fc in enumerate(fcs):
                    ws = slice(i * 128, (i + 1) * 128)
                    for tb in range(KTB):
                        pk = qps.tile([128, NT], F32, name="pk", tag="qkv")
                        for dc in range(DC):
                            nc.tensor.matmul(
                                pk[:], (wkp[dc][:, ws]),
                                (hT[dc][:, tb * NT:(tb + 1) * NT]),
                                start=(dc == 0), stop=(dc == DC - 1))
                        nc.vector.tensor_scalar_add(
                            kT[i][:, tb * NT:(tb + 1) * NT], pk[:], bks[i][:])
                    for tb in range(2):
                        pq = qps.tile([128, NT], F32, name="pq", tag="qkv")
                        for dc in range(DC):
                            nc.tensor.matmul(
                                pq[:], (wqp[dc][:, ws]),
                                (hT[dc][:, tb * NT:(tb + 1) * NT]),
                                start=(dc == 0), stop=(dc == DC - 1))
                        nc.vector.tensor_scalar_add(
                            qT[i][:, tb * NT:(tb + 1) * NT], pq[:], bqs[i][:])
                # --- V (token-major) ---
                bvr = pp.tile([1, FCP * 130], F32, name="bvr", tag="bvr")
                nc.sync.dma_start(bvr[:], bv_d.ap()[p])
                bvb = pp.tile([128, FCP * 130], F32, name="bvb", tag="bvb")
                nc.gpsimd.partition_broadcast(bvb[:], bvr[:])
                wvs = []
                for dc in range(DC):
                    wvt = wvp.tile([128, FCP * 130], F32R,
                                   name=f"wv{dc}", tag=f"wv{dc}")
                    nc.gpsimd.dma_start(wvt[:], wv_d.ap()[p, dc].bitcast(F32R))
                    wvs.append(wvt)
                vt = [vp.tile([128, FCP * 130], F32R, name=f"v{ti}",
                              tag=f"v{ti}") for ti in range(S)]
                for ti in range(S):
                    pv = qps.tile([128, FCP * 130], F32, name="pv", tag="qkv")
                    for dc in range(DC):
                        nc.tensor.matmul(
                            pv[:], (hT[dc][:, ti * 128:(ti + 1) * 128]),
                            (wvs[dc][:]),
                            start=(dc == 0), stop=(dc == DC - 1))
                    nc.vector.tensor_add(vt[ti][:], pv[:], bvb[:])
                # --- attention per head pair ---
                for i, fc in enumerate(fcs):
                    for qb in range(2):
                        if qb == 0:
                            schunks = list(range(S))
                        else:
                            schunks = list(range(QBC, 3 * QBC))
                        ctx_ps = [cps.tile([65, NT], F32, name=f"ctx{hh}",
                                           tag=f"ctx{hh}") for hh in range(2)]
                        nsc = len(schunks)
                        for idx, sc in enumerate(schunks):
                            # mask: (kind, index); kind: 0=none,1=tri,2=scalar
                            if qb == 0:
                                if sc < QBC:
                                    mk = (1, sc)
                                elif sc >= S - QBC:
                                    mk = (2, sc - (S - QBC))
                                else:
                                    mk = (0, 0)
                            else:
                                if sc < 2 * QBC:
                                    mk = (1, sc - QBC)
                                else:
                                    mk = (2, QBC + (sc - 2 * QBC))
                            # diagonal chunks only need columns >= 128*j
                            # (floored so the moving dim stays >= 256, where
                            # float32r runs at full PE rate)
                            coff = min(mk[1] * 128, NT - 256) \
                                if mk[0] == 1 else 0
                            coff = max(coff, 0)
                            ncols = NT - coff
                            sps_t = sps.tile([128, 2, NT], F32,
                                             name="sc", tag="sc")
                            e2 = ep.tile([128, 2, NT], F32R, name="e", tag="e")
                            for hh in range(2):
                                rows = slice(hh * HDf, (hh + 1) * HDf)
                                nc.tensor.matmul(
                                    sps_t[:, hh, coff:],
                                    (kT[i][rows, sc * 128:(sc + 1) * 128]),
                                    (qT[i][rows, qb * NT + coff:
                                           (qb + 1) * NT]),
                                    start=True, stop=True,
                                    tile_position=(hh * HDf, 0))
                            ebias = cms[:, mk[1]:mk[1] + 1] \
                                if mk[0] == 2 else zbias[:]
                            nc.scalar.activation(
                                e2[:, :, coff:], sps_t[:, :, coff:],
                                mybir.ActivationFunctionType.Exp,
                                bias=ebias)
                            if mk[0] == 1:
                                nc.vector.tensor_mul(
                                    e2[:, :, coff:], e2[:, :, coff:],
                                    tri[mk[1]][:, coff:].unsqueeze(1)
                                    .to_broadcast([128, 2, ncols]))
                            for hh in range(2):
                                nc.tensor.matmul(
                                    ctx_ps[hh][:, coff:],
                                    (vt[sc][:, (i * 2 + hh) * 65:
                                             (i * 2 + hh) * 65 + 65]),
                                    (e2[:, hh, coff:]),
                                    start=(idx == 0), stop=(idx == nsc - 1),
                                    skip_group_check=True)
                        for hh in range(2):
                            zrow = zp.tile([1, NT], F32, name="zrow",
                                           tag="zrow")
                            nc.vector.tensor_copy(zrow[:], ctx_ps[hh][64:65, :])
                            rz = zp.tile([1, NT], F32, name="rz", tag="rz")
                            nc.vector.reciprocal(rz[:], zrow[:])
                            zb = zbp.tile([64, NT], F32, name="zb", tag="zb")
                            nc.gpsimd.partition_broadcast(zb[:], rz[:])
                            rows = slice(hh * HDf, (hh + 1) * HDf)
                            nc.vector.tensor_mul(
                                ctxT[fc][rows, qb * NT:(qb + 1) * NT],
                                ctx_ps[hh][0:64, :], zb[:])

        hT_stack.close()

        # ---------------- Phase 3: projection + fused LN2 ---------------
        h2_stack = contextlib.ExitStack()
        h2p = h2_stack.enter_context(tc.tile_pool(name="h2Tp", bufs=1))
        h2T = [h2p.tile([128, TOWN], F32R, name=f"h2T{dc}", tag=f"h2T{dc}")
               for dc in range(DC)]
        with tc.tile_pool(name="proj_sb", bufs=1) as prp, \
             tc.tile_pool(name="proj_x", bufs=3) as pxp, \
             tc.tile_pool(name="proj_o", bufs=4) as pop, \
             tc.tile_pool(name="ln2s", bufs=8) as lsp2, \
             tc.tile_pool(name="ln2h", bufs=4) as lph2, \
             tc.tile_pool(name="projps", bufs=3, space=bass.MemorySpace.PSUM) as pps, \
             tc.tile_pool(name="ln2p", bufs=4, space=bass.MemorySpace.PSUM) as lpp2:
            bo_row = prp.tile([1, Dm], F32, name="bo_row", tag="bo_row")
            nc.sync.dma_start(bo_row[:], bo_d.ap())
            bob = prp.tile([128, Dm], F32, name="bob", tag="bob")
            nc.gpsimd.partition_broadcast(bob[:], bo_row[:])

            wos = []
            for fc in range(FC):
                wot = prp.tile([128, Dm], F32R, name=f"wo{fc}", tag=f"wo{fc}")
                nc.gpsimd.dma_start(
                    wot[:], wo_d.ap()[fc * 128:(fc + 1) * 128, :].bitcast(F32R))
                wos.append(wot)
            for ti in range(TOC):
                xo = pxp.tile([128, Dm], F32, name="xo", tag="xo")
                nc.sync.dma_start(xo[:], xr[ti])
                x1t = pop.tile([128, Dm], F32, name="x1t", tag="x1t")
                for oc in range(OC):
                    ppt = pps.tile([128, NO], F32, name="ppt", tag="ppt")
                    for fc in range(FC):
                        nc.tensor.matmul(
                            ppt[:],
                            (ctxT[fc][:, ti * 128:(ti + 1) * 128]),
                            (wos[fc][:, oc * NO:(oc + 1) * NO]),
                            start=(fc == 0), stop=(fc == FC - 1))
                    cols = slice(oc * NO, (oc + 1) * NO)
                    nc.vector.tensor_add(x1t[:, cols], ppt[:], xo[:, cols])
                    nc.vector.tensor_add(x1t[:, cols], x1t[:, cols],
                                         bob[:, cols])
                # fused LN2 on the freshly built x1 tile
                nsub = max(1, Dm // 512)
                st6 = lsp2.tile([128, nsub, 6], F32, name="st6b", tag="st6b")
                for sb_i in range(nsub):
                    nc.vector.bn_stats(
                        st6[:, sb_i, :],
                        x1t[:, sb_i * (Dm // nsub):(sb_i + 1) * (Dm // nsub)])
                agg = lsp2.tile([128, 2], F32, name="aggb", tag="aggb")
                nc.vector.bn_aggr(agg[:], st6[:])
                veps = lsp2.tile([128, 1], F32, name="vepsb", tag="vepsb")
                nc.vector.tensor_scalar_add(veps[:], agg[:, 1:2], EPS)
                std = lsp2.tile([128, 1], F32, name="stdb", tag="stdb")
                nc.scalar.sqrt(std[:], veps[:])
                rstd = lsp2.tile([128, 1], F32, name="rstdb", tag="rstdb")
                nc.vector.reciprocal(rstd[:], std[:])
                hb = lph2.tile([128, Dm], F32, name="hb", tag="hb")
                nc.vector.tensor_scalar(hb[:], x1t[:], agg[:, 0:1], rstd[:],
                                        op0=mybir.AluOpType.subtract,
                                        op1=mybir.AluOpType.mult)
                for dc in range(DC):
                    ps2 = lpp2.tile([128, 128], F32, name="tps2", tag="tps2")
                    nc.tensor.transpose(ps2[:], hb[:, dc * 128:(dc + 1) * 128],
                                        ident[:])
                    nc.scalar.copy(
                        h2T[dc][:, ti * 128:(ti + 1) * 128], ps2[:])
                nc.sync.dma_start(x1r[ti], x1t[:])

        # (LN2 is fused into the projection loop above; h2T ready here.)
        if False:
            for ti in range(TOC):
                nsub = max(1, Dm // 512)
                pass

        # ---------------- Phase 5: FFN ----------------------------------
        with tc.tile_pool(name="ffn_sb", bufs=1) as fp, \
             tc.tile_pool(name="ffn_w1", bufs=1) as w1p, \
             tc.tile_pool(name="ffn_w2", bufs=8) as w2p, \
             tc.tile_pool(name="ffn_b1", bufs=4) as b1p, \
             tc.tile_pool(name="ffn_x1", bufs=1) as fxp, \
             tc.tile_pool(name="ffn_out", bufs=2) as fop, \
             tc.tile_pool(name="ffps", bufs=3, space=bass.MemorySpace.PSUM) as fps, \
             tc.tile_pool(name="outps", bufs=3, space=bass.MemorySpace.PSUM) as ops:
            b2b = fp.tile([128, Dm], F32, name="b2b", tag="b2b")
            nc.sync.dma_start(b2b[:], b2_d.ap())
            oacc = [fp.tile([128, Dm], F32, name=f"oacc{ti}", tag=f"oacc{ti}")
                    for ti in range(TOC)]
            ffT = [fp.tile([128, TOWN], F32R, name=f"ffT{j}", tag=f"ffT{j}")
                   for j in range(GFC)]
            for g in range(NG):
                w1g = []
                for dc in range(DC):
                    w1t = w1p.tile([128, DFFm // NG], F32R,
                                   name=f"w1g{dc}", tag=f"w1g{dc}")
                    nc.gpsimd.dma_start(w1t[:], w1_d.ap()[g, dc].bitcast(F32R))
                    w1g.append(w1t)
                for j in range(GFC):
                    gf = g * GFC + j
                    b1t = b1p.tile([128, 1], F32, name="b1t", tag="b1t")
                    nc.gpsimd.dma_start(b1t[:], b1_d.ap()[gf])
                    for tb in range(TOWN // NT):
                        fpt = fps.tile([128, NT], F32, name="fpt", tag="fpt")
                        for dc in range(DC):
                            nc.tensor.matmul(
                                fpt[:], (w1g[dc][:, j * 128:(j + 1) * 128]),
                                (h2T[dc][:, tb * NT:(tb + 1) * NT]),
                                start=(dc == 0), stop=(dc == DC - 1))
                        nc.scalar.activation(
                            ffT[j][:, tb * NT:(tb + 1) * NT], fpt[:],
                            mybir.ActivationFunctionType.Relu,
                            bias=b1t[:])
                w2s = []
                for j in range(GFC):
                    gf = g * GFC + j
                    w2t = w2p.tile([128, Dm], F32R, name="w2t", tag="w2t")
                    nc.gpsimd.dma_start(
                        w2t[:], w2_d.ap()[gf * 128:(gf + 1) * 128, :]
                        .bitcast(F32R))
                    w2s.append(w2t)
                for ti in range(TOC):
                    x1t = None
                    if g == NG - 1:
                        x1t = fxp.tile([128, Dm], F32, name="x1f", tag="x1f")
                        nc.sync.dma_start(x1t[:], x1r[ti])
                    for oc in range(OC):
                        opt = ops.tile([128, NO], F32, name="opt", tag="opt")
                        for j in range(GFC):
                            nc.tensor.matmul(
                                opt[:],
                                (ffT[j][:, ti * 128:(ti + 1) * 128]),
                                (w2s[j][:, oc * NO:(oc + 1) * NO]),
                                start=(j == 0), stop=(j == GFC - 1))
                        cols = slice(oc * NO, (oc + 1) * NO)
                        if g == 0:
                            nc.vector.tensor_copy(oacc[ti][:, cols], opt[:])
                        elif g < NG - 1:
                            nc.vector.tensor_add(oacc[ti][:, cols],
                                                 oacc[ti][:, cols], opt[:])
                        else:
                            nc.vector.tensor_add(oacc[ti][:, cols],
                                                 oacc[ti][:, cols], opt[:])
                            nc.vector.tensor_add(oacc[ti][:, cols],
                                                 oacc[ti][:, cols],
                                                 x1t[:, cols])
                            ot = fop.tile([128, NO], F32, name="ot", tag="ot")
                            nc.vector.tensor_add(ot[:], oacc[ti][:, cols],
                                                 b2b[:, cols])
                            nc.sync.dma_start(outr[ti][:, cols], ot[:])
        h2_stack.close()
        ctx_stack.close()
    nc.compile()
    return nc


# ---------------------------------------------------------------------------
# host-side input preparation
# ---------------------------------------------------------------------------

def prepare_shared(cfg, Wq, Wk, Wv, Wo, bo, W1, b1, W2, b2, g1, be1, g2, be2):
    c = derive(cfg)
    Dm, Hn, DFFm, FC = cfg["D"], cfg["H"], cfg["DFF"], c["FC"]
    scale = 1.0 / np.sqrt(HD)
    wq_f = np.ascontiguousarray(Wq.transpose(1, 0, 2).reshape(Dm, Hn * HD))
    wk_f = np.ascontiguousarray(Wk.transpose(1, 0, 2).reshape(Dm, Hn * HD))
    wv_f = np.ascontiguousarray(Wv.transpose(1, 0, 2).reshape(Dm, Hn * HD))
    wq_e = (g1[:, None] * wq_f) * scale
    wk_e = g1[:, None] * wk_f
    wv_e = g1[:, None] * wv_f
    bq = ((be1 @ wq_f) * scale).reshape(FC, 128, 1)
    bk = (be1 @ wk_f).reshape(FC, 128, 1)
    bv = (be1 @ wv_f).reshape(1, Hn * HD)
    w1_e = g2[:, None] * W1
    b1_e = (b1 + be2 @ W1).reshape(DFFm // 128, 128, 1)
    DC, NPASS, FCP, NG = c["DC"], c["NPASS"], c["FCP"], cfg["NG"]

    def qkv_tile(w):
        # [D, F] -> [NPASS, DC, 128, FCP*128]
        return w.reshape(DC, 128, NPASS, FCP * 128).transpose(2, 0, 1, 3)

    # v weights get a zero column appended per head; its bias is 1.0, so the
    # v tiles come out of the matmul+bias with a built-in ones column that
    # accumulates the softmax normalizer during the ctx matmul.
    nheads = FCP * 2
    wv_r = wv_e.reshape(DC, 128, NPASS, nheads, HD)
    wv_a = np.concatenate(
        [wv_r, np.zeros((DC, 128, NPASS, nheads, 1), wv_r.dtype)], axis=-1)
    wv_t = wv_a.transpose(2, 0, 1, 3, 4).reshape(NPASS, DC, 128, nheads * 65)
    bv_r = bv.reshape(NPASS, nheads, HD)
    bv_a = np.concatenate(
        [bv_r, np.ones((NPASS, nheads, 1), bv_r.dtype)], axis=-1)
    bv_t = bv_a.reshape(NPASS, 1, nheads * 65)

    w1_t = w1_e.reshape(DC, 128, NG, DFFm // NG).transpose(2, 0, 1, 3)
    f32c = lambda a: np.ascontiguousarray(a, dtype=np.float32)
    return dict(
        wq=f32c(qkv_tile(wq_e)), wk=f32c(qkv_tile(wk_e)),
        wv=f32c(wv_t), bv=f32c(bv_t),
        bq=f32c(bq), bk=f32c(bk),
        wo=f32c(Wo), bo=f32c(bo.reshape(1, Dm)),
        w1=f32c(w1_t), b1=f32c(b1_e),
        w2=f32c(W2), b2=f32c(np.broadcast_to(b2.reshape(1, Dm), (128, Dm))),
        ident=np.eye(128, dtype=np.float32),
        ones=np.ones((128, 1), np.float32),
        zeros=np.zeros((128, 1), np.float32),
    )


def core_plan(cfg, half):
    """Return (perm, qposA, qposB) token index arrays for one core."""
    QB = cfg["QB"]
    Tf = cfg["T"]
    nb = Tf // QB  # 4 blocks
    if half == 0:
        bA, bB = nb - 1, 0
    else:
        bA, bB = nb - 2, 1
    own = {bA, bB}
    restA = [b for b in range(nb) if b not in own and b < bA]
    restB = [b for b in range(nb) if b not in own and b >= bA]
    blocks = [bA, bB] + restA + restB
    perm = np.concatenate([np.arange(b * QB, (b + 1) * QB) for b in blocks])
    qposA = np.arange(bA * QB, (bA + 1) * QB)
    qposB = np.arange(bB * QB, (bB + 1) * QB)
    return perm, qposA, qposB


def make_masks(cfg, perm, qposA, qposB):
    """tri tiles [QBC,128,NT]; whole-chunk exp-bias scalars (0 / -80)."""
    c = derive(cfg)
    QBC, NT, S = c["QBC"], c["NT"], c["S"]
    key = perm
    tri = np.zeros((QBC, 128, NT), np.float32)
    for j in range(QBC):
        ks = key[j * 128:(j + 1) * 128]
        tri[j] = (ks[:, None] <= qposA[None, :]).astype(np.float32)
    cm = np.zeros((2 * QBC, 128, 1), np.float32)
    for j in range(QBC):
        sc = S - QBC + j
        ks = key[sc * 128:(sc + 1) * 128]
        m = ks[:, None] <= qposA[None, :]
        assert m.all() or not m.any(), "chunk not homogeneous"
        cm[j] = 0.0 if m.all() else -80.0
    for j in range(QBC):
        sc = 2 * QBC + j
        ks = key[sc * 128:(sc + 1) * 128]
        m = ks[:, None] <= qposB[None, :]
        assert m.all() or not m.any(), "chunk not homogeneous"
        cm[QBC + j] = 0.0 if m.all() else -80.0
    return tri, cm


_NC_CACHE = {}

# test-harness knobs (ignored in normal grading use)
TRACE = False
TRACE_KWARGS = {}
LAST_RESULT = None


def _get_nc(key, cfg):
    if key not in _NC_CACHE:
        _NC_CACHE[key] = build(cfg)
    return _NC_CACHE[key]


def kernel(x, Wq, Wk, Wv, Wo, bo, W1, b1, W2, b2, g1, be1, g2, be2):
    cfg = FULL_CFG
    c = derive(cfg)
    x = np.asarray(x, np.float32)
    shared = prepare_shared(cfg, np.asarray(Wq), np.asarray(Wk), np.asarray(Wv),
                            np.asarray(Wo), np.asarray(bo), np.asarray(W1),
                            np.asarray(b1), np.asarray(W2), np.asarray(b2),
                            np.asarray(g1), np.asarray(be1), np.asarray(g2),
                            np.asarray(be2))
    nc = _get_nc("full", cfg)
    in_maps = []
    plans = []
    for core in range(N_CORES):
        b, half = core // 2, core % 2
        perm, qposA, qposB = core_plan(cfg, half)
        tri, cm = make_masks(cfg, perm, qposA, qposB)
        m = dict(shared)
        m["x"] = np.ascontiguousarray(x[b][perm], np.float32)
        m["tri"] = tri
        m["cm"] = cm
        in_maps.append(m)
        plans.append((b, perm))
    res = run_bass_kernel_spmd(nc, in_maps, list(range(N_CORES)),
                               trace=TRACE, **TRACE_KWARGS)
    global LAST_RESULT
    LAST_RESULT = res
    out = np.zeros((B, T, D), np.float32)
    TOWN = c["TOWN"]
    for core in range(N_CORES):
        b, perm = plans[core]
        o = res.results[core]["out"]
        out[b][perm[:TOWN]] = o
    return out


# revision 37
# speedup vs baseline: 1.8267x; 1.4763x over previous
"""Trainium2 Bass kernel for a dense pre-LN transformer block.

Sharding: 8 cores = 4 batches x 2 sequence-halves (zigzag query blocks).
Each core handles one batch element; K/V are computed redundantly for the
full sequence on both cores of a batch (cheaper than collectives), and each
core computes attention + proj + FFN for 1024 of the 2048 query tokens.

To keep the SPMD instruction stream identical across cores, each core's
tokens are host-side permuted to [own_blockA; own_blockB; rest] and all
causal-validity variation is carried in per-core mask data (triangular
tiles for diagonal blocks, per-partition 0/1 scalars for whole chunks).

All matmuls run as float32r (FP22, full PE rate); softmax/LN in fp32.
"""

import contextlib

import numpy as np

from concourse import bass, bacc, tile, mybir
from concourse.bass_utils import run_bass_kernel_spmd

F32 = mybir.dt.float32
F32R = mybir.dt.float32r
BF16 = mybir.dt.bfloat16

B, T, D = 4, 2048, 1024
H, HD = 16, 64
DFF = 4 * D
EPS = 1e-5
N_CORES = 8

FULL_CFG = dict(D=1024, H=16, T=2048, QB=512, DFF=4096, NG=4)
SMALL_CFG = dict(D=256, H=4, T=512, QB=128, DFF=512, NG=2)


def derive(cfg):
    c = dict(cfg)
    c["DC"] = cfg["D"] // 128            # d-chunks
    c["FC"] = cfg["H"] * HD // 128       # feature chunks (head pairs)
    c["FCP"] = 2                         # f-chunks per pass
    c["NPASS"] = c["FC"] // c["FCP"]
    c["S"] = cfg["T"] // 128             # key chunks
    c["QBC"] = cfg["QB"] // 128          # chunks per query block
    c["NT"] = cfg["QB"]                  # moving-dim tile (== query block)
    c["TOWN"] = 2 * cfg["QB"]            # tokens owned per core
    c["TOC"] = c["TOWN"] // 128
    c["NO"] = min(512, cfg["D"])
    c["OC"] = cfg["D"] // c["NO"]        # dout chunks of <=512
    c["GFC"] = (cfg["DFF"] // cfg["NG"]) // 128  # f-chunks per FFN group
    c["KTB"] = cfg["T"] // c["NT"]       # t-blocks for k over full T
    return c


def build(cfg):
    """Emit the bass program for one core. Returns nc."""
    c = derive(cfg)
    Dm, Tf, DFFm, NG = cfg["D"], cfg["T"], cfg["DFF"], cfg["NG"]
    DC, FC, FCP, NPASS = c["DC"], c["FC"], c["FCP"], c["NPASS"]
    S, QBC, NT, TOWN, TOC = c["S"], c["QBC"], c["NT"], c["TOWN"], c["TOC"]
    OC, NO, GFC, KTB = c["OC"], c["NO"], c["GFC"], c["KTB"]
    HDf = HD  # 64

    nc = bacc.Bacc("TRN2", target_bir_lowering=False, debug=False)

    # ---- DRAM I/O ----
    x_d = nc.dram_tensor("x", [Tf, Dm], F32, kind="ExternalInput")
    wq_d = nc.dram_tensor("wq", [NPASS, DC, 128, FCP * 128], F32,
                          kind="ExternalInput")
    wk_d = nc.dram_tensor("wk", [NPASS, DC, 128, FCP * 128], F32,
                          kind="ExternalInput")
    wv_d = nc.dram_tensor("wv", [NPASS, DC, 128, FCP * 130], F32,
                          kind="ExternalInput")
    bq_d = nc.dram_tensor("bq", [FC, 128, 1], F32, kind="ExternalInput")
    bk_d = nc.dram_tensor("bk", [FC, 128, 1], F32, kind="ExternalInput")
    bv_d = nc.dram_tensor("bv", [NPASS, 1, FCP * 130], F32,
                          kind="ExternalInput")
    wo_d = nc.dram_tensor("wo", [FC * 128, Dm], F32, kind="ExternalInput")
    bo_d = nc.dram_tensor("bo", [1, Dm], F32, kind="ExternalInput")
    w1_d = nc.dram_tensor("w1", [NG, DC, 128, DFFm // NG], F32,
                          kind="ExternalInput")
    b1_d = nc.dram_tensor("b1", [DFFm // 128, 128, 1], F32, kind="ExternalInput")
    w2_d = nc.dram_tensor("w2", [DFFm, Dm], F32, kind="ExternalInput")
    b2_d = nc.dram_tensor("b2", [128, Dm], F32, kind="ExternalInput")
    tri_d = nc.dram_tensor("tri", [QBC, 128, NT], F32, kind="ExternalInput")
    cm_d = nc.dram_tensor("cm", [2 * QBC, 128, 1], F32, kind="ExternalInput")
    idn_d = nc.dram_tensor("ident", [128, 128], F32, kind="ExternalInput")
    one_d = nc.dram_tensor("ones", [128, 1], F32, kind="ExternalInput")
    zro_d = nc.dram_tensor("zeros", [128, 1], F32, kind="ExternalInput")
    out_d = nc.dram_tensor("out", [TOWN, Dm], F32, kind="ExternalOutput")
    x1s_d = nc.dram_tensor("x1s", [TOWN, Dm], F32)  # internal scratch

    xr = x_d.ap().rearrange("(n p) d -> n p d", p=128)
    x1r = x1s_d.ap().rearrange("(n p) d -> n p d", p=128)
    outr = out_d.ap().rearrange("(n p) d -> n p d", p=128)

    with tile.TileContext(nc) as tc, contextlib.ExitStack() as top:
        cpool = top.enter_context(tc.tile_pool(name="const", bufs=1))
        ident = cpool.tile([128, 128], F32, name="ident", tag="ident")
        nc.sync.dma_start(ident[:], idn_d.ap())
        onesc = cpool.tile([128, 1], F32R, name="onesc", tag="onesc")
        nc.sync.dma_start(onesc[:], one_d.ap().bitcast(F32R))
        cms = cpool.tile([128, 2 * QBC], F32, name="cms", tag="cms")
        for i in range(2 * QBC):
            nc.gpsimd.dma_start(cms[:, i:i + 1], cm_d.ap()[i])
        zbias = cpool.tile([128, 1], F32, name="zbias", tag="zbias")
        nc.gpsimd.dma_start(zbias[:], zro_d.ap())

        ctx_stack = contextlib.ExitStack()
        ctxp = ctx_stack.enter_context(tc.tile_pool(name="ctxTp", bufs=1))
        ctxT = [ctxp.tile([128, TOWN], F32R, name=f"ctxT{fc}", tag=f"ctxT{fc}")
                for fc in range(FC)]

        hT_stack = contextlib.ExitStack()
        hp = hT_stack.enter_context(tc.tile_pool(name="hTp", bufs=1))
        hT = [hp.tile([128, Tf], F32R, name=f"hT{dc}", tag=f"hT{dc}")
              for dc in range(DC)]
        trip = hT_stack.enter_context(tc.tile_pool(name="trip", bufs=1))
        tri = []
        for i in range(QBC):
            m = trip.tile([128, NT], F32R, name=f"tri{i}", tag=f"tri{i}")
            nc.gpsimd.dma_start(m[:], tri_d.ap()[i].bitcast(F32R))
            tri.append(m)

        # ---------------- Phase 1: LN1 + transpose -> hT ----------------
        with tc.tile_pool(name="ln1", bufs=4) as lp, \
             tc.tile_pool(name="ln1s", bufs=8) as lsp, \
             tc.tile_pool(name="ln1p", bufs=6, space=bass.MemorySpace.PSUM) as lpp:
            for ti in range(S):
                xt = lp.tile([128, Dm], F32, name="xt", tag="xt")
                nsub = max(1, Dm // 512)
                st6 = lsp.tile([128, nsub, 6], F32, name="st6", tag="st6")
                for sb_i in range(nsub):
                    cs = slice(sb_i * (Dm // nsub), (sb_i + 1) * (Dm // nsub))
                    nc.sync.dma_start(xt[:, cs], xr[ti][:, cs])
                    nc.vector.bn_stats(st6[:, sb_i, :], xt[:, cs])
                agg = lsp.tile([128, 2], F32, name="agg", tag="agg")
                nc.vector.bn_aggr(agg[:], st6[:])
                veps = lsp.tile([128, 1], F32, name="veps",
                                  tag="veps")
                nc.vector.tensor_scalar_add(veps[:], agg[:, 1:2], EPS)
                std = lsp.tile([128, 1], F32, name="std", tag="std")
                nc.scalar.sqrt(std[:], veps[:])
                rstd = lsp.tile([128, 1], F32, name="rstd", tag="rstd")
                nc.vector.reciprocal(rstd[:], std[:])
                ht = lp.tile([128, Dm], F32, name="ht", tag="ht")
                hstep = Dm // nsub
                for sb_i in range(nsub):
                    cs = slice(sb_i * hstep, (sb_i + 1) * hstep)
                    nc.vector.tensor_scalar(ht[:, cs], xt[:, cs],
                                            agg[:, 0:1], rstd[:],
                                            op0=mybir.AluOpType.subtract,
                                            op1=mybir.AluOpType.mult)
                    for dc in range(sb_i * DC // nsub,
                                    (sb_i + 1) * DC // nsub):
                        ps = lpp.tile([128, 128], F32, name="tps", tag="tps")
                        nc.tensor.transpose(
                            ps[:], ht[:, dc * 128:(dc + 1) * 128], ident[:])
                        if dc % 2 == 0:
                            nc.scalar.copy(
                                hT[dc][:, ti * 128:(ti + 1) * 128], ps[:])
                        else:
                            nc.vector.tensor_copy(
                                hT[dc][:, ti * 128:(ti + 1) * 128], ps[:])

        # ---------------- Phase 2: per-pass QKV + attention -------------
        with tc.tile_pool(name="pass_sb", bufs=1) as pp, \
             tc.tile_pool(name="vtp", bufs=1) as vp, \
             tc.tile_pool(name="wvres", bufs=1) as wvp, \
             tc.tile_pool(name="expp", bufs=4) as ep, \
             tc.tile_pool(name="zrowp", bufs=2) as zp, \
             tc.tile_pool(name="zbp", bufs=2) as zbp, \
             tc.tile_pool(name="qkvps", bufs=2, space=bass.MemorySpace.PSUM) as qps, \
             tc.tile_pool(name="scps", bufs=2, space=bass.MemorySpace.PSUM) as sps, \
             tc.tile_pool(name="ctxps", bufs=1, space=bass.MemorySpace.PSUM) as cps:
            for p in range(NPASS):
                fcs = [p * FCP + i for i in range(FCP)]
                # --- K^T and Q^T (feature-major) ---
                kT = [pp.tile([128, Tf], F32R, name=f"kT{i}", tag=f"kT{i}")
                      for i in range(FCP)]
                qT = [pp.tile([128, 2 * NT], F32R, name=f"qT{i}", tag=f"qT{i}")
                      for i in range(FCP)]
                bks = []
                bqs = []
                for i, fc in enumerate(fcs):
                    bkt = pp.tile([128, 1], F32, name=f"bk{i}", tag=f"bk{i}")
                    nc.sync.dma_start(bkt[:], bk_d.ap()[fc])
                    bks.append(bkt)
                    bqt = pp.tile([128, 1], F32, name=f"bq{i}", tag=f"bq{i}")
                    nc.sync.dma_start(bqt[:], bq_d.ap()[fc])
                    bqs.append(bqt)
                wkp = []
                wqp = []
                for dc in range(DC):
                    wkt = pp.tile([128, FCP * 128], F32R,
                                  name=f"wkp{dc}", tag=f"wkp{dc}")
                    nc.gpsimd.dma_start(wkt[:], wk_d.ap()[p, dc].bitcast(F32R))
                    wkp.append(wkt)
                    wqt = pp.tile([128, FCP * 128], F32R,
                                  name=f"wqp{dc}", tag=f"wqp{dc}")
                    nc.gpsimd.dma_start(wqt[:], wq_d.ap()[p, dc].bitcast(F32R))
                    wqp.append(wqt)
                for i, fc in enumerate(fcs):
                    ws = slice(i * 128, (i + 1) * 128)
                    for tb in range(KTB):
                        pk = qps.tile([128, NT], F32, name="pk", tag="qkv")
                        for dc in range(DC):
                            nc.tensor.matmul(
                                pk[:], (wkp[dc][:, ws]),
                                (hT[dc][:, tb * NT:(tb + 1) * NT]),
                                start=(dc == 0), stop=(dc == DC - 1))
                        nc.vector.tensor_scalar_add(
                            kT[i][:, tb * NT:(tb + 1) * NT], pk[:], bks[i][:])
                    for tb in range(2):
                        pq = qps.tile([128, NT], F32, name="pq", tag="qkv")
                        for dc in range(DC):
                            nc.tensor.matmul(
                                pq[:], (wqp[dc][:, ws]),
                                (hT[dc][:, tb * NT:(tb + 1) * NT]),
                                start=(dc == 0), stop=(dc == DC - 1))
                        nc.vector.tensor_scalar_add(
                            qT[i][:, tb * NT:(tb + 1) * NT], pq[:], bqs[i][:])
                # --- V (token-major) ---
                bvr = pp.tile([1, FCP * 130], F32, name="bvr", tag="bvr")
                nc.sync.dma_start(bvr[:], bv_d.ap()[p])
                bvb = pp.tile([128, FCP * 130], F32, name="bvb", tag="bvb")
                nc.gpsimd.partition_broadcast(bvb[:], bvr[:])
                wvs = []
                for dc in range(DC):
                    wvt = wvp.tile([128, FCP * 130], F32R,
                                   name=f"wv{dc}", tag=f"wv{dc}")
                    nc.gpsimd.dma_start(wvt[:], wv_d.ap()[p, dc].bitcast(F32R))
                    wvs.append(wvt)
                vt = [vp.tile([128, FCP * 130], F32R, name=f"v{ti}",
                              tag=f"v{ti}") for ti in range(S)]
                for ti in range(S):
                    pv = qps.tile([128, FCP * 130], F32, name="pv", tag="qkv")
                    for dc in range(DC):
                        nc.tensor.matmul(
                            pv[:], (hT[dc][:, ti * 128:(ti + 1) * 128]),
                            (wvs[dc][:]),
                            start=(dc == 0), stop=(dc == DC - 1))
                    nc.vector.tensor_add(vt[ti][:], pv[:], bvb[:])
                # --- attention per head pair ---
                for i, fc in enumerate(fcs):
                    for qb in range(2):
                        if qb == 0:
                            schunks = list(range(S))
                        else:
                            schunks = list(range(QBC, 3 * QBC))
                        ctx_ps = [cps.tile([65, NT], F32, name=f"ctx{hh}",
                                           tag=f"ctx{hh}") for hh in range(2)]
                        nsc = len(schunks)
                        for idx, sc in enumerate(schunks):
                            # mask: (kind, index); kind: 0=none,1=tri,2=scalar
                            if qb == 0:
                                if sc < QBC:
                                    mk = (1, sc)
                                elif sc >= S - QBC:
                                    mk = (2, sc - (S - QBC))
                                else:
                                    mk = (0, 0)
                            else:
                                if sc < 2 * QBC:
                                    mk = (1, sc - QBC)
                                else:
                                    mk = (2, QBC + (sc - 2 * QBC))
                            # diagonal chunks only need columns >= 128*j
                            # (floored so the moving dim stays >= 256, where
                            # float32r runs at full PE rate)
                            coff = min(mk[1] * 128, NT - 256) \
                                if mk[0] == 1 else 0
                            coff = max(coff, 0)
                            ncols = NT - coff
                            sps_t = sps.tile([128, 2, NT], F32,
                                             name="sc", tag="sc")
                            e2 = ep.tile([128, 2, NT], F32R, name="e", tag="e")
                            for hh in range(2):
                                rows = slice(hh * HDf, (hh + 1) * HDf)
                                nc.tensor.matmul(
                                    sps_t[:, hh, coff:],
                                    (kT[i][rows, sc * 128:(sc + 1) * 128]),
                                    (qT[i][rows, qb * NT + coff:
                                           (qb + 1) * NT]),
                                    start=True, stop=True,
                                    tile_position=(hh * HDf, 0))
                            ebias = cms[:, mk[1]:mk[1] + 1] \
                                if mk[0] == 2 else zbias[:]
                            nc.scalar.activation(
                                e2[:, :, coff:], sps_t[:, :, coff:],
                                mybir.ActivationFunctionType.Exp,
                                bias=ebias)
                            if mk[0] == 1:
                                nc.vector.tensor_mul(
                                    e2[:, :, coff:], e2[:, :, coff:],
                                    tri[mk[1]][:, coff:].unsqueeze(1)
                                    .to_broadcast([128, 2, ncols]))
                            for hh in range(2):
                                nc.tensor.matmul(
                                    ctx_ps[hh][:, coff:],
                                    (vt[sc][:, (i * 2 + hh) * 65:
                                             (i * 2 + hh) * 65 + 65]),
                                    (e2[:, hh, coff:]),
                                    start=(idx == 0), stop=(idx == nsc - 1),
                                    skip_group_check=True)
                        for hh in range(2):
                            zrow = zp.tile([1, NT], F32, name="zrow",
                                           tag="zrow")
                            nc.vector.tensor_copy(zrow[:], ctx_ps[hh][64:65, :])
                            rz = zp.tile([1, NT], F32, name="rz", tag="rz")
                            nc.vector.reciprocal(rz[:], zrow[:])
                            zb = zbp.tile([64, NT], F32, name="zb", tag="zb")
                            nc.gpsimd.partition_broadcast(zb[:], rz[:])
                            rows = slice(hh * HDf, (hh + 1) * HDf)
                            nc.vector.tensor_mul(
                                ctxT[fc][rows, qb * NT:(qb + 1) * NT],
                                ctx_ps[hh][0:64, :], zb[:])

        hT_stack.close()

        # ---------------- Phase 3: projection + fused LN2 ---------------
        h2_stack = contextlib.ExitStack()
        h2p = h2_stack.enter_context(tc.tile_pool(name="h2Tp", bufs=1))
        h2T = [h2p.tile([128, TOWN], F32R, name=f"h2T{dc}", tag=f"h2T{dc}")
               for dc in range(DC)]
        with tc.tile_pool(name="proj_sb", bufs=1) as prp, \
             tc.tile_pool(name="proj_x", bufs=3) as pxp, \
             tc.tile_pool(name="proj_o", bufs=4) as pop, \
             tc.tile_pool(name="ln2s", bufs=8) as lsp2, \
             tc.tile_pool(name="ln2h", bufs=4) as lph2, \
             tc.tile_pool(name="projps", bufs=4, space=bass.MemorySpace.PSUM) as pps, \
             tc.tile_pool(name="ln2p", bufs=4, space=bass.MemorySpace.PSUM) as lpp2:
            bo_row = prp.tile([1, Dm], F32, name="bo_row", tag="bo_row")
            nc.sync.dma_start(bo_row[:], bo_d.ap())
            bob = prp.tile([128, Dm], F32, name="bob", tag="bob")
            nc.gpsimd.partition_broadcast(bob[:], bo_row[:])

            wos = []
            for fc in range(FC):
                wot = prp.tile([128, Dm], F32R, name=f"wo{fc}", tag=f"wo{fc}")
                nc.gpsimd.dma_start(
                    wot[:], wo_d.ap()[fc * 128:(fc + 1) * 128, :].bitcast(F32R))
                wos.append(wot)
            for ti in range(TOC):
                xo = pxp.tile([128, Dm], F32, name="xo", tag="xo")
                nc.sync.dma_start(xo[:], xr[ti])
                x1t = pop.tile([128, Dm], F32, name="x1t", tag="x1t")
                for oc in range(OC):
                    ppt = pps.tile([128, NO], F32, name="ppt", tag="ppt")
                    for fc in range(FC):
                        nc.tensor.matmul(
                            ppt[:],
                            (ctxT[fc][:, ti * 128:(ti + 1) * 128]),
                            (wos[fc][:, oc * NO:(oc + 1) * NO]),
                            start=(fc == 0), stop=(fc == FC - 1))
                    cols = slice(oc * NO, (oc + 1) * NO)
                    nc.vector.tensor_add(x1t[:, cols], ppt[:], xo[:, cols])
                    nc.vector.tensor_add(x1t[:, cols], x1t[:, cols],
                                         bob[:, cols])
                # fused LN2 on the freshly built x1 tile
                nsub = max(1, Dm // 512)
                st6 = lsp2.tile([128, nsub, 6], F32, name="st6b", tag="st6b")
                for sb_i in range(nsub):
                    nc.vector.bn_stats(
                        st6[:, sb_i, :],
                        x1t[:, sb_i * (Dm // nsub):(sb_i + 1) * (Dm // nsub)])
                agg = lsp2.tile([128, 2], F32, name="aggb", tag="aggb")
                nc.vector.bn_aggr(agg[:], st6[:])
                veps = lsp2.tile([128, 1], F32, name="vepsb", tag="vepsb")
                nc.vector.tensor_scalar_add(veps[:], agg[:, 1:2], EPS)
                std = lsp2.tile([128, 1], F32, name="stdb", tag="stdb")
                nc.scalar.sqrt(std[:], veps[:])
                rstd = lsp2.tile([128, 1], F32, name="rstdb", tag="rstdb")
                nc.vector.reciprocal(rstd[:], std[:])
                hb = lph2.tile([128, Dm], F32, name="hb", tag="hb")
                nc.vector.tensor_scalar(hb[:], x1t[:], agg[:, 0:1], rstd[:],
                                        op0=mybir.AluOpType.subtract,
                                        op1=mybir.AluOpType.mult)
                for dc in range(DC):
                    ps2 = lpp2.tile([128, 128], F32, name="tps2", tag="tps2")
                    nc.tensor.transpose(ps2[:], hb[:, dc * 128:(dc + 1) * 128],
                                        ident[:])
                    nc.scalar.copy(
                        h2T[dc][:, ti * 128:(ti + 1) * 128], ps2[:])
                nc.sync.dma_start(x1r[ti], x1t[:])

        # (LN2 is fused into the projection loop above; h2T ready here.)
        if False:
            for ti in range(TOC):
                nsub = max(1, Dm // 512)
                pass

        # ---------------- Phase 5: FFN ----------------------------------
        with tc.tile_pool(name="ffn_sb", bufs=1) as fp, \
             tc.tile_pool(name="ffn_w1", bufs=1) as w1p, \
             tc.tile_pool(name="ffn_w2", bufs=8) as w2p, \
             tc.tile_pool(name="ffn_b1", bufs=4) as b1p, \
             tc.tile_pool(name="ffn_x1", bufs=1) as fxp, \
             tc.tile_pool(name="ffn_out", bufs=2) as fop, \
             tc.tile_pool(name="ffps", bufs=3, space=bass.MemorySpace.PSUM) as fps, \
             tc.tile_pool(name="outps", bufs=3, space=bass.MemorySpace.PSUM) as ops:
            b2b = fp.tile([128, Dm], F32, name="b2b", tag="b2b")
            nc.sync.dma_start(b2b[:], b2_d.ap())
            oacc = [fp.tile([128, Dm], F32, name=f"oacc{ti}", tag=f"oacc{ti}")
                    for ti in range(TOC)]
            ffT = [fp.tile([128, TOWN], F32R, name=f"ffT{j}", tag=f"ffT{j}")
                   for j in range(GFC)]
            for g in range(NG):
                w1g = []
                for dc in range(DC):
                    w1t = w1p.tile([128, DFFm // NG], F32R,
                                   name=f"w1g{dc}", tag=f"w1g{dc}")
                    nc.gpsimd.dma_start(w1t[:], w1_d.ap()[g, dc].bitcast(F32R))
                    w1g.append(w1t)
                for j in range(GFC):
                    gf = g * GFC + j
                    b1t = b1p.tile([128, 1], F32, name="b1t", tag="b1t")
                    nc.gpsimd.dma_start(b1t[:], b1_d.ap()[gf])
                    for tb in range(TOWN // NT):
                        fpt = fps.tile([128, NT], F32, name="fpt", tag="fpt")
                        for dc in range(DC):
                            nc.tensor.matmul(
                                fpt[:], (w1g[dc][:, j * 128:(j + 1) * 128]),
                                (h2T[dc][:, tb * NT:(tb + 1) * NT]),
                                start=(dc == 0), stop=(dc == DC - 1))
                        nc.scalar.activation(
                            ffT[j][:, tb * NT:(tb + 1) * NT], fpt[:],
                            mybir.ActivationFunctionType.Relu,
                            bias=b1t[:])
                w2s = []
                for j in range(GFC):
                    gf = g * GFC + j
                    w2t = w2p.tile([128, Dm], F32R, name="w2t", tag="w2t")
                    nc.gpsimd.dma_start(
                        w2t[:], w2_d.ap()[gf * 128:(gf + 1) * 128, :]
                        .bitcast(F32R))
                    w2s.append(w2t)
                for ti in range(TOC):
                    x1t = None
                    if g == NG - 1:
                        x1t = fxp.tile([128, Dm], F32, name="x1f", tag="x1f")
                        nc.sync.dma_start(x1t[:], x1r[ti])
                    for oc in range(OC):
                        opt = ops.tile([128, NO], F32, name="opt", tag="opt")
                        for j in range(GFC):
                            nc.tensor.matmul(
                                opt[:],
                                (ffT[j][:, ti * 128:(ti + 1) * 128]),
                                (w2s[j][:, oc * NO:(oc + 1) * NO]),
                                start=(j == 0), stop=(j == GFC - 1))
                        cols = slice(oc * NO, (oc + 1) * NO)
                        if g == 0:
                            nc.vector.tensor_copy(oacc[ti][:, cols], opt[:])
                        elif g < NG - 1:
                            nc.vector.tensor_add(oacc[ti][:, cols],
                                                 oacc[ti][:, cols], opt[:])
                        else:
                            nc.vector.tensor_add(oacc[ti][:, cols],
                                                 oacc[ti][:, cols], opt[:])
                            nc.vector.tensor_add(oacc[ti][:, cols],
                                                 oacc[ti][:, cols],
                                                 x1t[:, cols])
                            ot = fop.tile([128, NO], F32, name="ot", tag="ot")
                            nc.vector.tensor_add(ot[:], oacc[ti][:, cols],
                                                 b2b[:, cols])
                            nc.sync.dma_start(outr[ti][:, cols], ot[:])
        h2_stack.close()
        ctx_stack.close()
    nc.compile()
    return nc


# ---------------------------------------------------------------------------
# host-side input preparation
# ---------------------------------------------------------------------------

def prepare_shared(cfg, Wq, Wk, Wv, Wo, bo, W1, b1, W2, b2, g1, be1, g2, be2):
    c = derive(cfg)
    Dm, Hn, DFFm, FC = cfg["D"], cfg["H"], cfg["DFF"], c["FC"]
    scale = 1.0 / np.sqrt(HD)
    wq_f = np.ascontiguousarray(Wq.transpose(1, 0, 2).reshape(Dm, Hn * HD))
    wk_f = np.ascontiguousarray(Wk.transpose(1, 0, 2).reshape(Dm, Hn * HD))
    wv_f = np.ascontiguousarray(Wv.transpose(1, 0, 2).reshape(Dm, Hn * HD))
    wq_e = (g1[:, None] * wq_f) * scale
    wk_e = g1[:, None] * wk_f
    wv_e = g1[:, None] * wv_f
    bq = ((be1 @ wq_f) * scale).reshape(FC, 128, 1)
    bk = (be1 @ wk_f).reshape(FC, 128, 1)
    bv = (be1 @ wv_f).reshape(1, Hn * HD)
    w1_e = g2[:, None] * W1
    b1_e = (b1 + be2 @ W1).reshape(DFFm // 128, 128, 1)
    DC, NPASS, FCP, NG = c["DC"], c["NPASS"], c["FCP"], cfg["NG"]

    def qkv_tile(w):
        # [D, F] -> [NPASS, DC, 128, FCP*128]
        return w.reshape(DC, 128, NPASS, FCP * 128).transpose(2, 0, 1, 3)

    # v weights get a zero column appended per head; its bias is 1.0, so the
    # v tiles come out of the matmul+bias with a built-in ones column that
    # accumulates the softmax normalizer during the ctx matmul.
    nheads = FCP * 2
    wv_r = wv_e.reshape(DC, 128, NPASS, nheads, HD)
    wv_a = np.concatenate(
        [wv_r, np.zeros((DC, 128, NPASS, nheads, 1), wv_r.dtype)], axis=-1)
    wv_t = wv_a.transpose(2, 0, 1, 3, 4).reshape(NPASS, DC, 128, nheads * 65)
    bv_r = bv.reshape(NPASS, nheads, HD)
    bv_a = np.concatenate(
        [bv_r, np.ones((NPASS, nheads, 1), bv_r.dtype)], axis=-1)
    bv_t = bv_a.reshape(NPASS, 1, nheads * 65)

    w1_t = w1_e.reshape(DC, 128, NG, DFFm // NG).transpose(2, 0, 1, 3)
    f32c = lambda a: np.ascontiguousarray(a, dtype=np.float32)
    return dict(
        wq=f32c(qkv_tile(wq_e)), wk=f32c(qkv_tile(wk_e)),
        wv=f32c(wv_t), bv=f32c(bv_t),
        bq=f32c(bq), bk=f32c(bk),
        wo=f32c(Wo), bo=f32c(bo.reshape(1, Dm)),
        w1=f32c(w1_t), b1=f32c(b1_e),
        w2=f32c(W2), b2=f32c(np.broadcast_to(b2.reshape(1, Dm), (128, Dm))),
        ident=np.eye(128, dtype=np.float32),
        ones=np.ones((128, 1), np.float32),
        zeros=np.zeros((128, 1), np.float32),
    )


def core_plan(cfg, half):
    """Return (perm, qposA, qposB) token index arrays for one core."""
    QB = cfg["QB"]
    Tf = cfg["T"]
    nb = Tf // QB  # 4 blocks
    if half == 0:
        bA, bB = nb - 1, 0
    else:
        bA, bB = nb - 2, 1
    own = {bA, bB}
    restA = [b for b in range(nb) if b not in own and b < bA]
    restB = [b for b in range(nb) if b not in own and b >= bA]
    blocks = [bA, bB] + restA + restB
    perm = np.concatenate([np.arange(b * QB, (b + 1) * QB) for b in blocks])
    qposA = np.arange(bA * QB, (bA + 1) * QB)
    qposB = np.arange(bB * QB, (bB + 1) * QB)
    return perm, qposA, qposB


def make_masks(cfg, perm, qposA, qposB):
    """tri tiles [QBC,128,NT]; whole-chunk exp-bias scalars (0 / -80)."""
    c = derive(cfg)
    QBC, NT, S = c["QBC"], c["NT"], c["S"]
    key = perm
    tri = np.zeros((QBC, 128, NT), np.float32)
    for j in range(QBC):
        ks = key[j * 128:(j + 1) * 128]
        tri[j] = (ks[:, None] <= qposA[None, :]).astype(np.float32)
    cm = np.zeros((2 * QBC, 128, 1), np.float32)
    for j in range(QBC):
        sc = S - QBC + j
        ks = key[sc * 128:(sc + 1) * 128]
        m = ks[:, None] <= qposA[None, :]
        assert m.all() or not m.any(), "chunk not homogeneous"
        cm[j] = 0.0 if m.all() else -80.0
    for j in range(QBC):
        sc = 2 * QBC + j
        ks = key[sc * 128:(sc + 1) * 128]
        m = ks[:, None] <= qposB[None, :]
        assert m.all() or not m.any(), "chunk not homogeneous"
        cm[QBC + j] = 0.0 if m.all() else -80.0
    return tri, cm


_NC_CACHE = {}

# test-harness knobs (ignored in normal grading use)
TRACE = False
TRACE_KWARGS = {}
LAST_RESULT = None


def _get_nc(key, cfg):
    if key not in _NC_CACHE:
        _NC_CACHE[key] = build(cfg)
    return _NC_CACHE[key]


def kernel(x, Wq, Wk, Wv, Wo, bo, W1, b1, W2, b2, g1, be1, g2, be2):
    cfg = FULL_CFG
    c = derive(cfg)
    x = np.asarray(x, np.float32)
    shared = prepare_shared(cfg, np.asarray(Wq), np.asarray(Wk), np.asarray(Wv),
                            np.asarray(Wo), np.asarray(bo), np.asarray(W1),
                            np.asarray(b1), np.asarray(W2), np.asarray(b2),
                            np.asarray(g1), np.asarray(be1), np.asarray(g2),
                            np.asarray(be2))
    nc = _get_nc("full", cfg)
    in_maps = []
    plans = []
    for core in range(N_CORES):
        b, half = core // 2, core % 2
        perm, qposA, qposB = core_plan(cfg, half)
        tri, cm = make_masks(cfg, perm, qposA, qposB)
        m = dict(shared)
        m["x"] = np.ascontiguousarray(x[b][perm], np.float32)
        m["tri"] = tri
        m["cm"] = cm
        in_maps.append(m)
        plans.append((b, perm))
    res = run_bass_kernel_spmd(nc, in_maps, list(range(N_CORES)),
                               trace=TRACE, **TRACE_KWARGS)
    global LAST_RESULT
    LAST_RESULT = res
    out = np.zeros((B, T, D), np.float32)
    TOWN = c["TOWN"]
    for core in range(N_CORES):
        b, perm = plans[core]
        o = res.results[core]["out"]
        out[b][perm[:TOWN]] = o
    return out
